# revision 1
# baseline (speedup 1.0000x reference)
"""DetectionBEVLoss Trainium2 kernel: 8-core data-parallel (1 batch/core).

Layout: per core 65536 elements as [128 partitions, 512 free]. Host packs all
inputs into one fp16 array [128, 32, 512] per core (slot map below). Rotated
IoU uses a branch-free Liang-Barsky edge-clip formulation (each quad's edges
clipped against the other box in that box's axis-aligned frame; boundary line
integral x dy - y dx is rotation invariant, evaluated in the target frame).
"""
import math

import ml_dtypes
import numpy as np

import concourse.bacc as bacc
import concourse.bass as bass
import concourse.mybir as mybir
import concourse.tile as tile
from concourse.bass_utils import run_bass_kernel_spmd

F16 = mybir.dt.float16
F32 = mybir.dt.float32
OP = mybir.AluOpType
AF = mybir.ActivationFunctionType

P = 128          # partitions
FW = 512         # free width per partition (128*512 = 65536 elems/core)
NCH = 2          # free-dim chunks
FC = FW // NCH   # chunk width

# slot map in the packed fp16 input [128, 32, 512]
# 0-8: reg_pred c0..c8 | 9-17: reg_targets c0..c8 | 18: iou_pred | 19: iou_targets
# 20: cls_targets (as f16) | 21: reg_weights (as f16) | 22-31: cls_pred c0..c9
NSLOT = 32

EPS = 1e-7


def _ap(t, s0, slot_dims, col0, ncol, colstep=1):
    """Manual AP into tile t ([128, S, W]): base slot s0, then
    (slot_step, count) dims, innermost column dim. Slot stride taken
    from the tile's own AP (W elements)."""
    ss = t.ap[-2][0]
    ap = [list(t.ap[0])] + [[s * ss, c] for s, c in slot_dims] + [[colstep, ncol]]
    return bass.AP(tensor=t.tensor, offset=t.offset + s0 * ss + col0, ap=ap)


def build_bass():
    nc = bacc.Bacc("TRN2", target_bir_lowering=False, debug=False)
    h16 = nc.declare_dram_parameter("h16", [P, NSLOT, FW], F16, isOutput=False)
    outp = nc.declare_dram_parameter("out", [1, 32], F32, isOutput=True)

    with tile.TileContext(nc) as tc:
        with (
            tc.tile_pool(name="main", bufs=1) as pool,
            tc.tile_pool(name="small", bufs=1) as spool,
            tc.tile_pool(name="ps", bufs=1, space="PSUM") as ppool,
        ):
            IN = pool.tile([P, NSLOT, FW], F16)
            # DMA in: geometry slots first, cls last
            nc.sync.dma_start(out=IN[:, 0:22, :], in_=h16[:, 0:22, :])
            nc.sync.dma_start(out=IN[:, 22:32, :], in_=h16[:, 22:32, :])

            pibias = spool.tile([P, 1], F32)
            nc.vector.memset(pibias, math.pi / 2)
            ones = spool.tile([P, 1], F32)
            nc.vector.memset(ones, 1.0)
            ACC = spool.tile([P, 32], F32)
            nc.vector.memset(ACC, 0.0)

            # ---- full-width trig / halves / cd-sd / dxy ----
            # sin/cos via Taylor poly on DVE (yaw in [0,1); ACT's sin table
            # can't share a table-set with exp/ln)
            TR = pool.tile([P, 4, FW], F16)   # cosp sinp cost sint
            X2 = pool.tile([P, 2, FW], F16)   # yaw^2 for p and t
            YAWS = _ap(IN, 6, [(9, 2)], 0, FW)  # slots 6, 15
            nc.vector.tensor_tensor(out=X2, in0=YAWS, in1=YAWS, op=OP.mult)
            SPH = pool.tile([P, 2, FW], F16)
            nc.vector.tensor_scalar(out=SPH, in0=X2, scalar1=1.0 / 120,
                                    scalar2=-1.0 / 6, op0=OP.mult, op1=OP.add)
            nc.vector.tensor_tensor(out=SPH, in0=SPH, in1=X2, op=OP.mult)
            nc.vector.scalar_tensor_tensor(out=_ap(TR, 1, [(2, 2)], 0, FW), in0=SPH,
                                           scalar=1.0, in1=YAWS, op0=OP.add, op1=OP.mult)
            CPH = pool.tile([P, 2, FW], F16)
            nc.vector.tensor_scalar(out=CPH, in0=X2, scalar1=-1.0 / 720,
                                    scalar2=1.0 / 24, op0=OP.mult, op1=OP.add)
            nc.vector.tensor_tensor(out=CPH, in0=CPH, in1=X2, op=OP.mult)
            nc.vector.tensor_scalar(out=CPH, in0=CPH, scalar1=-0.5,
                                    scalar2=None, op0=OP.add)
            nc.vector.tensor_tensor(out=CPH, in0=CPH, in1=X2, op=OP.mult)
            nc.vector.tensor_scalar(out=_ap(TR, 0, [(2, 2)], 0, FW), in0=CPH,
                                    scalar1=1.0, scalar2=None, op0=OP.add)

            HV = pool.tile([P, 4, FW], F16)   # lht wht lhp whp
            # IN slots 12,13 = [wht,lht]*2 -> write reversed into HV slots 1,0
            nc.vector.tensor_scalar(
                out=_ap(HV, 1, [(-1, 2)], 0, FW), in0=IN[:, 12:14, :],
                scalar1=0.5, scalar2=None, op0=OP.mult)
            nc.vector.tensor_scalar(
                out=_ap(HV, 3, [(-1, 2)], 0, FW), in0=IN[:, 3:5, :],
                scalar1=0.5, scalar2=None, op0=OP.mult)

            CS = pool.tile([P, 2, FW], F16)   # cd sd
            TP = pool.tile([P, 2, FW], F16)
            TQ = pool.tile([P, 2, FW], F16)
            # TP = [cp*ct, sp*st]
            nc.vector.tensor_tensor(out=TP, in0=TR[:, 0:2, :], in1=TR[:, 2:4, :], op=OP.mult)
            # TQ = [sp*ct, cp*st]  (in0 = TR slots [1,0])
            nc.vector.tensor_tensor(out=TQ, in0=_ap(TR, 1, [(-1, 2)], 0, FW),
                                    in1=TR[:, 2:4, :], op=OP.mult)
            nc.vector.tensor_tensor(out=CS[:, 0, :], in0=TP[:, 0, :], in1=TP[:, 1, :], op=OP.add)
            nc.vector.tensor_tensor(out=CS[:, 1, :], in0=TQ[:, 0, :], in1=TQ[:, 1, :], op=OP.subtract)

            DXY = pool.tile([P, 2, FW], F16)  # dx dy
            nc.vector.tensor_tensor(out=DXY, in0=IN[:, 0:2, :], in1=IN[:, 9:11, :], op=OP.subtract)

            ACS = pool.tile([P, 4, FW], F16)  # |cp| |sp| |ct| |st|
            nc.scalar.activation(ACS, TR, AF.Abs)

            for j in range(NCH):
                c0 = j * FC
                cols = slice(c0, c0 + FC)

                def inp(s):
                    return IN[:, s, cols]

                def hv(s):
                    return HV[:, s, cols]

                # ---------- corner transforms ----------
                DC = pool.tile([P, 4, FC], F16, tag="DC")  # dcxA dcyA dcxB dcyB
                # PTall = [ct*dx, ct*dy, cp*dx, cp*dy]; QTall = [st*..., sp*...]
                PTall = pool.tile([P, 4, FC], F16, tag="PT")
                QTall = pool.tile([P, 4, FC], F16, tag="QT")
                nc.vector.tensor_tensor(out=PTall,
                                        in0=_ap(DXY, 0, [(0, 2), (1, 2)], c0, FC),
                                        in1=_ap(TR, 2, [(-2, 2), (0, 2)], c0, FC), op=OP.mult)
                nc.vector.tensor_tensor(out=QTall,
                                        in0=_ap(DXY, 0, [(0, 2), (1, 2)], c0, FC),
                                        in1=_ap(TR, 3, [(-2, 2), (0, 2)], c0, FC), op=OP.mult)
                # dcx = c*dx + s*dy ; dcy = c*dy - s*dx  (both directions at once)
                nc.vector.tensor_tensor(out=_ap(DC, 0, [(2, 2)], 0, FC),
                                        in0=_ap(PTall, 0, [(2, 2)], 0, FC),
                                        in1=_ap(QTall, 1, [(2, 2)], 0, FC), op=OP.add)
                nc.vector.tensor_tensor(out=_ap(DC, 1, [(2, 2)], 0, FC),
                                        in0=_ap(PTall, 1, [(2, 2)], 0, FC),
                                        in1=_ap(QTall, 0, [(2, 2)], 0, FC), op=OP.subtract)

                # UVX: cd*[lhp,whp,lht,wht], sd*[whp,lhp,wht,lht]
                UVX = pool.tile([P, 8, FC], F16, tag="UV")
                nc.vector.tensor_tensor(out=UVX[:, 0:4, :],
                                        in0=_ap(CS, 0, [(0, 4)], c0, FC),
                                        in1=_ap(HV, 2, [(-2, 2), (1, 2)], c0, FC), op=OP.mult)
                nc.vector.tensor_tensor(out=UVX[:, 4:8, :],
                                        in0=_ap(CS, 1, [(0, 4)], c0, FC),
                                        in1=_ap(HV, 3, [(-1, 4)], c0, FC), op=OP.mult)
                # SC layout: [sA, sB, sD, sC, pB, pA, pC, pD]
                SC = pool.tile([P, 8, FC], F16, tag="SC")
                nc.vector.tensor_tensor(out=_ap(SC, 0, [(2, 4)], 0, FC),
                                        in0=_ap(UVX, 0, [(2, 2), (5, 2)], 0, FC),
                                        in1=_ap(UVX, 4, [(2, 2), (-3, 2)], 0, FC), op=OP.add)
                nc.vector.tensor_tensor(out=_ap(SC, 1, [(2, 4)], 0, FC),
                                        in0=_ap(UVX, 0, [(2, 2), (5, 2)], 0, FC),
                                        in1=_ap(UVX, 4, [(2, 2), (-3, 2)], 0, FC), op=OP.subtract)

                # corners: slots 0-3 AX, 4-7 AY, 8-11 BX, 12-15 BY  (CW order)
                # AX = dcx + [sA,-sB,-sA,sB] ; AY = dcy + [sC,-sD,-sC,sD]
                # BX = dcx2 + [-pA,pB,pA,-pB]; BY = dcy2 + [pC,-pD,-pC,pD]
                CRN = pool.tile([P, 16, FC], F16, tag="CRN")
                bcast = lambda src, n: _ap(src[0], src[1], [(0, n)], c0, FC)

                def corner2(dst0, step, dcslot, scslot, scstep, op):
                    # CRN[{dst0, dst0+step}] = DC[dcslot] op SC[{scslot, scslot+scstep}]
                    nc.vector.tensor_tensor(
                        out=_ap(CRN, dst0, [(step, 2)], 0, FC),
                        in0=_ap(DC, dcslot, [(0, 2)], 0, FC),
                        in1=_ap(SC, scslot, [(scstep, 2)], 0, FC), op=op)

                corner2(0, 3, 0, 0, 1, OP.add)        # AX0=dcx+sA, AX3=dcx+sB
                corner2(1, 1, 0, 1, -1, OP.subtract)  # AX1=dcx-sB, AX2=dcx-sA
                corner2(4, 3, 1, 3, -1, OP.add)       # AY0=dcy+sC, AY3=dcy+sD
                corner2(5, 1, 1, 2, 1, OP.subtract)   # AY1=dcy-sD, AY2=dcy-sC
                corner2(9, 1, 2, 4, 1, OP.add)        # BX1=dcx2+pB, BX2=dcx2+pA
                corner2(8, 3, 2, 5, -1, OP.subtract)  # BX0=dcx2-pA, BX3=dcx2-pB
                corner2(12, 3, 3, 6, 1, OP.add)       # BY0=dcy2+pC, BY3=dcy2+pD
                corner2(13, 1, 3, 7, -1, OP.subtract) # BY1=dcy2-pD, BY2=dcy2-pC

                # ---------- edge vectors, reciprocals (per 4-slot group) ----------
                # boxes are parallelograms: edge 2 = -edge 0, edge 3 = -edge 1,
                # so only edges 0,1 need the reciprocal; 2,3 are negated copies
                RD = pool.tile([P, 16, FC], F16, tag="RD")
                for g in range(4):
                    b = g * 4
                    D32g = pool.tile([P, 2, FC], F32, tag="D32g")
                    nc.vector.tensor_tensor(out=D32g, in0=CRN[:, b + 1:b + 3, :],
                                            in1=CRN[:, b:b + 2, :], op=OP.subtract)
                    # keep D away from exact 0: fp16 corners cancel exactly for
                    # near-parallel edges; approx reciprocal of 0 is NaN
                    nc.vector.tensor_scalar(out=D32g, in0=D32g, scalar1=1e-12,
                                            scalar2=None, op0=OP.add)
                    R32g = pool.tile([P, 2, FC], F32, tag="R32g")
                    nc.vector.reciprocal_approx_fast(out=R32g.rearrange("p a b -> p (a b)"),
                                                     in_=D32g.rearrange("p a b -> p (a b)"))
                    nc.vector.tensor_scalar(out=RD[:, b:b + 2, :], in0=R32g,
                                            scalar1=-8000.0, scalar2=8000.0,
                                            op0=OP.max, op1=OP.min)
                    nc.vector.tensor_scalar(out=RD[:, b + 2:b + 4, :], in0=RD[:, b:b + 2, :],
                                            scalar1=-1.0, scalar2=None, op0=OP.mult)

                # ---------- Liang-Barsky slab clip ----------
                # slot groups: 0-3 use L=lht(HV0), 4-7 wht(HV1), 8-11 lhp(HV2), 12-15 whp(HV3)
                # lo = -(L|r| + C r), hi = L|r| - C r  (r clamped finite -> no NaN)
                # |r| and L*|r| identical for opposite edges: compute on 8 slots,
                # read back through a repeat-AP
                RA = pool.tile([P, 4, 2, FC], F16, tag="RA8")
                nc.scalar.activation(RA, _ap(RD, 0, [(4, 4), (1, 2)], 0, FC), AF.Abs)
                Q1 = pool.tile([P, 16, FC], F16, tag="NB")
                nc.vector.tensor_tensor(out=Q1, in0=CRN, in1=RD, op=OP.mult)   # C*r
                RL = pool.tile([P, 4, 2, FC], F16, tag="RL8")
                nc.vector.tensor_tensor(out=RL, in0=_ap(HV, 0, [(1, 4), (0, 2)], c0, FC),
                                        in1=RA, op=OP.mult)                    # L*|r|
                RLrep = _ap(RL, 0, [(2, 4), (0, 2), (1, 2)], 0, FC)
                HI = pool.tile([P, 16, FC], F16, tag="NA")
                nc.vector.tensor_tensor(out=_ap(HI, 0, [(4, 4), (2, 2), (1, 2)], 0, FC),
                                        in0=RLrep,
                                        in1=_ap(Q1, 0, [(4, 4), (2, 2), (1, 2)], 0, FC),
                                        op=OP.subtract)
                TQ2 = pool.tile([P, 16, FC], F16, tag="P2")
                nc.vector.tensor_tensor(out=_ap(TQ2, 0, [(4, 4), (2, 2), (1, 2)], 0, FC),
                                        in0=RLrep,
                                        in1=_ap(Q1, 0, [(4, 4), (2, 2), (1, 2)], 0, FC),
                                        op=OP.add)                             # -lo
                # t0 = max(-min(tqx,tqy), 0) ; t1 = min(min(hix,hiy), 1)
                T0 = pool.tile([P, 8, FC], F16, tag="P1")
                T1 = pool.tile([P, 8, FC], F16, tag="NB")
                nc.vector.tensor_tensor(out=T0, in0=_ap(TQ2, 0, [(8, 2), (1, 4)], 0, FC),
                                        in1=_ap(TQ2, 4, [(8, 2), (1, 4)], 0, FC), op=OP.min)
                nc.vector.tensor_scalar(out=T0, in0=T0, scalar1=-1.0, scalar2=0.0,
                                        op0=OP.mult, op1=OP.max)
                nc.vector.tensor_tensor(out=T1, in0=_ap(HI, 0, [(8, 2), (1, 4)], 0, FC),
                                        in1=_ap(HI, 4, [(8, 2), (1, 4)], 0, FC), op=OP.min)
                nc.vector.tensor_scalar(out=T1, in0=T1, scalar1=1.0, scalar2=None, op0=OP.min)
                SEG = pool.tile([P, 8, FC], F16, tag="SEG")
                nc.vector.tensor_tensor(out=SEG, in0=T1, in1=T0, op=OP.subtract)
                nc.vector.tensor_scalar(out=SEG, in0=SEG, scalar1=0.0, scalar2=None, op0=OP.max)

                # ---------- cross products (dir A) + accumulate intersection ----------
                CR1 = pool.tile([P, 4, FC], F16, tag="CR1")
                CR2 = pool.tile([P, 4, FC], F16, tag="CR2")
                nc.vector.tensor_tensor(out=CR1[:, 0:3, :], in0=CRN[:, 0:3, :],
                                        in1=CRN[:, 5:8, :], op=OP.mult)
                nc.vector.tensor_tensor(out=CR1[:, 3, :], in0=CRN[:, 3, :],
                                        in1=CRN[:, 4, :], op=OP.mult)
                nc.vector.tensor_tensor(out=CR2[:, 0:3, :], in0=CRN[:, 4:7, :],
                                        in1=CRN[:, 1:4, :], op=OP.mult)
                nc.vector.tensor_tensor(out=CR2[:, 3, :], in0=CRN[:, 7, :],
                                        in1=CRN[:, 0, :], op=OP.mult)
                nc.vector.tensor_tensor(out=CR1, in0=CR1, in1=CR2, op=OP.subtract)
                CA = pool.tile([P, 4, FC], F16, tag="CA")
                nc.vector.tensor_tensor(out=CA, in0=CR1, in1=SEG[:, 0:4, :], op=OP.mult)
                CAT = pool.tile([P, 2, FC], F16, tag="CAT")
                nc.vector.tensor_tensor(out=CAT, in0=CA[:, 0:2, :], in1=CA[:, 2:4, :], op=OP.add)
                ACA = pool.tile([P, FC], F32, tag="ACA")
                nc.vector.tensor_tensor(out=ACA, in0=CAT[:, 0, :], in1=CAT[:, 1, :], op=OP.add)
                SB2 = pool.tile([P, 2, FC], F16, tag="SB2")
                nc.vector.tensor_tensor(out=SB2, in0=SEG[:, 4:6, :], in1=SEG[:, 6:8, :], op=OP.add)
                SBS = pool.tile([P, FC], F16, tag="SBS")
                nc.vector.tensor_tensor(out=SBS, in0=SB2[:, 0, :], in1=SB2[:, 1, :], op=OP.add)
                M32 = pool.tile([P, FC], F32, tag="M32")
                nc.vector.tensor_tensor(out=M32, in0=hv(0), in1=hv(1), op=OP.mult)  # lht*wht
                MM = pool.tile([P, FC], F32, tag="MM")
                nc.vector.tensor_tensor(out=MM, in0=M32, in1=SBS, op=OP.mult)
                nc.vector.scalar_tensor_tensor(out=ACA, in0=MM, scalar=-2.0, in1=ACA,
                                               op0=OP.mult, op1=OP.add)

                INTER = pool.tile([P, FC], F32, tag="INTER")
                nc.scalar.activation(INTER, ACA, AF.Abs, scale=0.5)
                AP32 = pool.tile([P, FC], F32, tag="AP32")
                nc.vector.tensor_tensor(out=AP32, in0=hv(2), in1=hv(3), op=OP.mult)  # lhp*whp
                U1 = pool.tile([P, FC], F32, tag="U1")
                nc.vector.tensor_tensor(out=U1, in0=AP32, in1=M32, op=OP.add)
                UNION = pool.tile([P, FC], F32, tag="UNION")
                nc.vector.scalar_tensor_tensor(out=UNION, in0=U1, scalar=4.0, in1=INTER,
                                               op0=OP.mult, op1=OP.subtract)
                UC = pool.tile([P, FC], F32, tag="UC")
                nc.vector.tensor_scalar(out=UC, in0=UNION, scalar1=EPS, scalar2=None, op0=OP.max)
                RUC = pool.tile([P, FC], F32, tag="RUC")
                nc.vector.reciprocal_approx_fast(out=RUC, in_=UC)
                IOU = pool.tile([P, FC], F32, tag="IOU")
                nc.vector.tensor_tensor(out=IOU, in0=INTER, in1=RUC, op=OP.mult)
                MU = pool.tile([P, FC], F32, tag="MU")
                nc.vector.tensor_scalar(out=MU, in0=UNION, scalar1=EPS, scalar2=None, op0=OP.is_gt)
                nc.vector.tensor_tensor(out=IOU, in0=IOU, in1=MU, op=OP.mult)

                # ---------- enclosing box diag^2 + center dist (Pool engine) ----------
                PA_ = pool.tile([P, 4, FC], F16, tag="PA_")
                PB_ = pool.tile([P, 4, FC], F16, tag="PB_")
                # PA = [lhp|cp|, whp|sp|, lht|ct|, wht|st|] ; hv order [lht,wht,lhp,whp]
                nc.gpsimd.tensor_tensor(out=PA_, in0=_ap(HV, 2, [(-2, 2), (1, 2)], c0, FC),
                                        in1=ACS[:, :, cols], op=OP.mult)
                nc.gpsimd.tensor_tensor(out=PB_, in0=_ap(HV, 2, [(-2, 2), (1, 2)], c0, FC),
                                        in1=_ap(ACS, 1, [(2, 2), (-1, 2)], c0, FC), op=OP.mult)
                EX = pool.tile([P, 2, FC], F16, tag="EX")  # [ex_p, ex_t]
                EY = pool.tile([P, 2, FC], F16, tag="EY")
                nc.gpsimd.tensor_tensor(out=EX, in0=_ap(PA_, 0, [(2, 2)], 0, FC),
                                        in1=_ap(PA_, 1, [(2, 2)], 0, FC), op=OP.add)
                nc.gpsimd.tensor_tensor(out=EY, in0=_ap(PB_, 0, [(2, 2)], 0, FC),
                                        in1=_ap(PB_, 1, [(2, 2)], 0, FC), op=OP.add)
                PX = _ap(IN, 0, [(9, 2)], c0, FC)   # [xp, xt]
                PY = _ap(IN, 1, [(9, 2)], c0, FC)   # [yp, yt]
                XE = pool.tile([P, 2, FC], F16, tag="XE")
                XD = pool.tile([P, 2, FC], F16, tag="XD")
                YE = pool.tile([P, 2, FC], F16, tag="YE")
                YD = pool.tile([P, 2, FC], F16, tag="YD")
                nc.gpsimd.tensor_tensor(out=XE, in0=PX, in1=EX, op=OP.add)
                nc.gpsimd.tensor_tensor(out=XD, in0=PX, in1=EX, op=OP.subtract)
                nc.gpsimd.tensor_tensor(out=YE, in0=PY, in1=EY, op=OP.add)
                nc.gpsimd.tensor_tensor(out=YD, in0=PY, in1=EY, op=OP.subtract)
                HL = pool.tile([P, 4, FC], F16, tag="HL")  # hx lx hy ly
                nc.vector.tensor_tensor(out=HL[:, 0, :], in0=XE[:, 0, :], in1=XE[:, 1, :], op=OP.max)
                nc.vector.tensor_tensor(out=HL[:, 1, :], in0=XD[:, 0, :], in1=XD[:, 1, :], op=OP.min)
                nc.vector.tensor_tensor(out=HL[:, 2, :], in0=YE[:, 0, :], in1=YE[:, 1, :], op=OP.max)
                nc.vector.tensor_tensor(out=HL[:, 3, :], in0=YD[:, 0, :], in1=YD[:, 1, :], op=OP.min)
                W2 = pool.tile([P, 2, FC], F16, tag="W2")
                nc.gpsimd.tensor_tensor(out=W2, in0=_ap(HL, 0, [(2, 2)], 0, FC),
                                        in1=_ap(HL, 1, [(2, 2)], 0, FC), op=OP.subtract)
                SQ = pool.tile([P, 2, FC], F32, tag="SQ")
                nc.gpsimd.tensor_tensor(out=SQ, in0=W2, in1=W2, op=OP.mult)
                C2 = pool.tile([P, FC], F32, tag="C2")
                nc.gpsimd.tensor_tensor(out=C2, in0=SQ[:, 0, :], in1=SQ[:, 1, :], op=OP.add)
                nc.vector.tensor_scalar(out=C2, in0=C2, scalar1=EPS, scalar2=None, op0=OP.max)
                D2P = pool.tile([P, 2, FC], F32, tag="D2P")
                nc.gpsimd.tensor_tensor(out=D2P, in0=DXY[:, :, cols], in1=DXY[:, :, cols], op=OP.mult)
                D2 = pool.tile([P, FC], F32, tag="D2")
                nc.gpsimd.tensor_tensor(out=D2, in0=D2P[:, 0, :], in1=D2P[:, 1, :], op=OP.add)
                RC2 = pool.tile([P, FC], F32, tag="RC2")
                nc.vector.reciprocal_approx_fast(out=RC2, in_=C2)
                DL = pool.tile([P, FC], F32, tag="DL")
                nc.vector.tensor_tensor(out=DL, in0=D2, in1=RC2, op=OP.mult)
                nc.vector.tensor_tensor(out=DL, in0=DL, in1=IOU, op=OP.subtract)
                wmask = inp(21)
                PR32 = pool.tile([P, FC], F32, tag="PR32")
                nc.vector.tensor_tensor(out=PR32, in0=DL, in1=wmask, op=OP.mult)
                JK32 = pool.tile([P, FC], F32, tag="JK32")
                nc.scalar.activation(JK32, PR32, AF.Copy,
                                     accum_out=ACC[:, 2 + 16 * j:3 + 16 * j])

            # ---- full-width tail: smooth-L1, BCE, focal (independent of geometry) ----
            def inpF(s):
                return IN[:, s, :]

            # ---------- smooth L1 on z,h,vx,vy (Pool) ----------
            DD = pool.tile([P, 4, FW], F16, tag="UV")
            nc.gpsimd.tensor_tensor(out=DD[:, 0, :], in0=inpF(2), in1=inpF(11), op=OP.subtract)
            nc.gpsimd.tensor_tensor(out=DD[:, 1, :], in0=inpF(5), in1=inpF(14), op=OP.subtract)
            nc.gpsimd.tensor_tensor(out=DD[:, 2:4, :], in0=IN[:, 7:9, :],
                                    in1=IN[:, 16:18, :], op=OP.subtract)
            nc.scalar.activation(DD, DD, AF.Abs)
            SLM = pool.tile([P, 4, FW], F16, tag="SEG")
            nc.vector.tensor_scalar(out=SLM, in0=DD, scalar1=1.0, scalar2=None, op0=OP.is_lt)
            AM1 = pool.tile([P, 4, FW], F16, tag="RD")
            nc.vector.tensor_scalar(out=AM1, in0=DD, scalar1=-1.0, scalar2=None, op0=OP.add)
            nc.gpsimd.tensor_tensor(out=AM1, in0=AM1, in1=AM1, op=OP.mult)
            nc.vector.scalar_tensor_tensor(out=AM1, in0=SLM, scalar=0.5, in1=AM1,
                                           op0=OP.mult, op1=OP.mult)
            nc.gpsimd.tensor_tensor(out=DD, in0=DD, in1=AM1, op=OP.add)  # sl1 + 0.5
            PRS = pool.tile([P, 4, FW], F16, tag="CRN")
            nc.vector.tensor_tensor(out=PRS, in0=DD,
                                    in1=_ap(IN, 21, [(0, 4)], 0, FW), op=OP.mult)
            JK16 = pool.tile([P, FW], F16, tag="JK16")
            for k in range(4):
                nc.scalar.activation(JK16, PRS[:, k, :], AF.Copy,
                                     accum_out=ACC[:, 3 + k + 0:4 + k + 0])

            # ---------- BCE on iou head (Pool + ACT) ----------
            BR = pool.tile([P, FW], F16, tag="BR")
            nc.vector.tensor_scalar(out=BR, in0=inpF(18), scalar1=0.0, scalar2=None, op0=OP.max)
            BA = pool.tile([P, FW], F16, tag="BA")
            nc.scalar.activation(BA, inpF(18), AF.Abs)
            BS = pool.tile([P, FW], F16, tag="BS")
            nc.scalar.activation(BS, BA, AF.Exp, scale=-1.0)   # e^{-|x|}
            nc.scalar.activation(BS, BS, AF.Ln, bias=1.0)      # ln(1 + e^{-|x|})
            nc.gpsimd.tensor_tensor(out=BR, in0=BR, in1=BS, op=OP.add)
            BXY = pool.tile([P, FW], F16, tag="BXY")
            nc.gpsimd.tensor_tensor(out=BXY, in0=inpF(18), in1=inpF(19), op=OP.mult)
            nc.gpsimd.tensor_tensor(out=BR, in0=BR, in1=BXY, op=OP.subtract)
            PRB = pool.tile([P, FW], F16, tag="PRB")
            nc.vector.tensor_tensor(out=PRB, in0=BR, in1=inpF(21), op=OP.mult)
            nc.scalar.activation(JK16, PRB, AF.Copy,
                                 accum_out=ACC[:, 7 + 0:8 + 0])

            # ---------- focal ----------
            ET = pool.tile([P, 10, FW], F16, tag="NA")
            nc.scalar.activation(ET, IN[:, 22:32, :], AF.Exp)
            S5 = pool.tile([P, 5, FW], F16, tag="S5")
            nc.vector.tensor_tensor(out=S5, in0=ET[:, 0:5, :], in1=ET[:, 5:10, :], op=OP.add)
            S2 = pool.tile([P, 2, FW], F16, tag="S2")
            nc.vector.tensor_tensor(out=S2, in0=S5[:, 0:2, :], in1=S5[:, 2:4, :], op=OP.add)
            SS = pool.tile([P, FW], F16, tag="SS")
            nc.vector.tensor_tensor(out=SS, in0=S2[:, 0, :], in1=S2[:, 1, :], op=OP.add)
            nc.vector.tensor_tensor(out=SS, in0=SS, in1=S5[:, 4, :], op=OP.add)
            clsf = inpF(20)
            MT = pool.tile([P, 10, FW], F16, tag="NB")
            for c in range(10):
                nc.vector.scalar_tensor_tensor(out=MT[:, c, :], in0=clsf, scalar=float(c),
                                               in1=IN[:, 22 + c, :],
                                               op0=OP.is_equal, op1=OP.mult)
            nc.vector.tensor_tensor(out=S5, in0=MT[:, 0:5, :], in1=MT[:, 5:10, :], op=OP.add)
            nc.vector.tensor_tensor(out=S2, in0=S5[:, 0:2, :], in1=S5[:, 2:4, :], op=OP.add)
            LT = pool.tile([P, FW], F16, tag="LT")
            nc.vector.tensor_tensor(out=LT, in0=S2[:, 0, :], in1=S2[:, 1, :], op=OP.add)
            nc.vector.tensor_tensor(out=LT, in0=LT, in1=S5[:, 4, :], op=OP.add)
            LNS = pool.tile([P, FW], F16, tag="LNS")
            nc.scalar.activation(LNS, SS, AF.Ln)
            LPT = pool.tile([P, FW], F16, tag="LPT")
            nc.vector.tensor_tensor(out=LPT, in0=LT, in1=LNS, op=OP.subtract)
            PTT = pool.tile([P, FW], F16, tag="PTT")
            nc.scalar.activation(PTT, LPT, AF.Exp)
            ONEM = pool.tile([P, FW], F16, tag="ONEM")
            nc.vector.tensor_scalar(out=ONEM, in0=PTT, scalar1=-1.0, scalar2=1.0,
                                    op0=OP.mult, op1=OP.add)
            nc.vector.tensor_tensor(out=ONEM, in0=ONEM, in1=ONEM, op=OP.mult)
            MPOS = pool.tile([P, FW], F16, tag="MPOS")
            nc.vector.tensor_scalar(out=MPOS, in0=clsf, scalar1=0.5, scalar2=None, op0=OP.is_gt)
            nc.vector.tensor_scalar(out=MPOS, in0=MPOS, scalar1=-0.5, scalar2=0.75,
                                    op0=OP.mult, op1=OP.add)
            F1 = pool.tile([P, FW], F16, tag="F1")
            nc.vector.tensor_tensor(out=F1, in0=ONEM, in1=LPT, op=OP.mult)
            nc.vector.tensor_tensor(out=F1, in0=F1, in1=MPOS, op=OP.mult)
            VLD = pool.tile([P, FW], F16, tag="VLD")
            nc.vector.tensor_scalar(out=VLD, in0=clsf, scalar1=-0.5, scalar2=None, op0=OP.is_ge)
            PRF = pool.tile([P, FW], F16, tag="PRF")
            nc.vector.tensor_tensor(out=PRF, in0=F1, in1=VLD, op=OP.mult)
            nc.scalar.activation(JK16, PRF, AF.Copy, scale=-1.0,
                                 accum_out=ACC[:, 0 + 0:1 + 0])
            nc.scalar.activation(JK16, VLD, AF.Copy,
                                 accum_out=ACC[:, 1 + 0:2 + 0])
            nc.scalar.activation(JK16, inpF(21), AF.Copy,
                                 accum_out=ACC[:, 8 + 0:9 + 0])

            # ---------- cross-partition reduce + output ----------
            PS = ppool.tile([1, 32], F32)
            nc.tensor.matmul(PS, ones, ACC, start=True, stop=True)
            OUT = spool.tile([1, 32], F32)
            nc.scalar.copy(out=OUT, in_=PS)
            nc.sync.dma_start(out=outp[:, :], in_=OUT)
    nc.compile()
    return nc


_NC_CACHE = None


def _get_nc():
    global _NC_CACHE
    if _NC_CACHE is None:
        _NC_CACHE = build_bass()
    return _NC_CACHE


def pack_inputs(cls_pred, reg_pred, iou_pred, reg_targets, iou_targets,
                cls_targets, reg_weights):
    """Returns list of 8 per-core input dicts."""
    B = cls_pred.shape[0]
    maps = []
    for b in range(B):
        h = np.empty((NSLOT, P, FW), np.float16)
        h[0:9] = np.asarray(reg_pred[b], np.float32).reshape(9, P, FW)
        h[9:18] = np.asarray(reg_targets[b], np.float32).reshape(9, P, FW)
        h[18] = np.asarray(iou_pred[b], np.float32).reshape(P, FW)
        h[19] = np.asarray(iou_targets[b], np.float32).reshape(P, FW)
        h[20] = np.asarray(cls_targets[b]).astype(np.float32).reshape(P, FW)
        h[21] = np.asarray(reg_weights[b]).astype(np.float32).reshape(P, FW)
        h[22:32] = np.asarray(cls_pred[b], np.float32).reshape(10, P, FW)
        maps.append({"h16": np.ascontiguousarray(h.transpose(1, 0, 2))})
    return maps


def combine(parts):
    """parts: [8, 1, 32] per-core raw sums -> final [7] float32."""
    p = np.asarray(parts, np.float64).sum(0).reshape(2, 16).sum(0)
    focal_s, valid_s, diou_s, z_s, h_s, vx_s, vy_s, bce_s, w_s = p[:9]
    num_pos = max(w_s, 1.0)
    cls_loss = focal_s / max(valid_s, 1.0)
    bev_loss = (diou_s + w_s) / num_pos
    z_loss = (z_s - 0.5 * w_s) / num_pos
    h_loss = (h_s - 0.5 * w_s) / num_pos
    vel_loss = (vx_s + vy_s - w_s) / num_pos
    iou_loss = bce_s / num_pos
    total = cls_loss + 2.0 * bev_loss + z_loss + h_loss + vel_loss + iou_loss
    return np.array([total, cls_loss, bev_loss, z_loss, h_loss, vel_loss, iou_loss],
                    np.float32)


def kernel(cls_pred, reg_pred, iou_pred, reg_targets, iou_targets,
           cls_targets, reg_weights, _trace=False):
    # accept jax or numpy inputs
    cls_pred, reg_pred, iou_pred, reg_targets, iou_targets, cls_targets, reg_weights = (
        np.asarray(a) for a in (cls_pred, reg_pred, iou_pred, reg_targets,
                                iou_targets, cls_targets, reg_weights))
    nc = _get_nc()
    in_maps = pack_inputs(cls_pred, reg_pred, iou_pred, reg_targets,
                          iou_targets, cls_targets, reg_weights)
    res = run_bass_kernel_spmd(nc, in_maps, core_ids=list(range(8)), trace=_trace)
    parts = [res.results[i]["out"] for i in range(8)]
    out = combine(parts)
    if _trace:
        return out, res
    return out



# revision 8
# speedup vs baseline: 1.5328x; 1.5328x over previous
"""DetectionBEVLoss Trainium2 kernel: 8-core data-parallel (1 batch/core).

Layout: per core 65536 sites as [128 partitions, 512 free]. Host packs all
inputs into one fp16 array [128, 32, 512] per core (slot map below).

Rotated intersection via a single-direction Green's-theorem identity:
  area(A ∩ B) = sum over A's 4 edges of  int clamp_a(x(y)) dy
with the y-integral restricted to |y| <= b (B = target box, axis-aligned
in its own frame, half-dims a x b; A = pred box with corners in B's
frame). Each edge needs one y-interval clip and a clamp antiderivative
Phi(x) = c*x - c^2/2 at the two clipped endpoints; the only reciprocals
are 1/|cos dth| and 1/|sin dth| (clamped at 2^-6, with the perturbation
applied consistently to the corner geometry so it stays exact-for-a-
perturbed-box).

Trig runs on the Scalar (ACT) engine via the sin table (cos = sin(x+pi/2)).
Smooth-L1 uses sl1(d)+0.5 = 0.5 d^2 + 0.5 - 0.5 relu(|d|-1)^2 so the sums
are plain Square-accumulates on ACT. BCE uses softplus(x) - x*t.
"""
import math

import numpy as np

import concourse.bacc as bacc
import concourse.bass as bass
import concourse.mybir as mybir
import concourse.tile as tile
from concourse.bass_utils import run_bass_kernel_spmd

F16 = mybir.dt.float16
F32 = mybir.dt.float32
OP = mybir.AluOpType
AF = mybir.ActivationFunctionType

P = 128          # partitions
FW = 512         # free width per partition (128*512 = 65536 sites/core)

# slot map in the packed fp16 input [128, 32, 512]
# 0-8: reg_pred c0..c8 | 9-17: reg_targets c0..c8 | 18: iou_pred | 19: iou_targets
# 20: cls_targets (as f16) | 21: reg_weights (as f16) | 22-31: cls_pred c0..c9
NSLOT = 32

EPSC = 2.0 ** -6     # |cos|,|sin| clamp (slope guard)

# ACC slots (fp32 [P,32])
A_FOC, A_DIOU, A_ZSQ, A_ZRL, A_HSQ, A_HRL, A_VSQ, A_VRL, A_WSP, A_WIT, A_W = range(11)


def _ap(t, s0, slot_dims, col0=0, ncol=FW, colstep=1):
    """Manual AP into tile t ([128, S, W]): base slot s0, then
    (slot_step, count) dims, innermost column dim."""
    ss = t.ap[-2][0]
    ap = [list(t.ap[0])] + [[s * ss, c] for s, c in slot_dims] + [[colstep, ncol]]
    return bass.AP(tensor=t.tensor, offset=t.offset + s0 * ss + col0, ap=ap)


def build_bass():
    nc = bacc.Bacc("TRN2", target_bir_lowering=False, debug=False)
    h16 = nc.declare_dram_parameter("h16", [P, NSLOT, FW], F16, isOutput=False)
    outp = nc.declare_dram_parameter("out", [1, 32], F32, isOutput=True)

    with tile.TileContext(nc) as tc:
        with (
            tc.tile_pool(name="main", bufs=1) as pool,
            tc.tile_pool(name="small", bufs=1) as spool,
            tc.tile_pool(name="ps", bufs=1, space="PSUM") as ppool,
        ):
            IN = pool.tile([P, NSLOT, FW], F16)
            nc.sync.dma_start(out=IN[:, 0:22, :], in_=h16[:, 0:22, :])
            nc.sync.dma_start(out=IN[:, 22:32, :], in_=h16[:, 22:32, :])

            ones = spool.tile([P, 1], F32)
            nc.vector.memset(ones, 1.0)
            ACC = spool.tile([P, 32], F32)
            nc.vector.memset(ACC, 0.0)
            PIH = spool.tile([P, 1], F32)
            nc.vector.memset(PIH, math.pi / 2)
            NEG1 = spool.tile([P, 1], F32)
            nc.vector.memset(NEG1, -1.0)
            TINY = spool.tile([P, 1], F32)
            nc.vector.memset(TINY, 1e-6)

            def inp(s):
                return IN[:, s, :]

            def bc(t, s, n):  # broadcast slot s of tile t over n slots
                return _ap(t, s, [(0, n)])

            # ================= head ======================================
            DTH = pool.tile([P, FW], F16, tag="DTH")
            nc.vector.tensor_tensor(out=DTH, in0=inp(6), in1=inp(15), op=OP.subtract)

            # ACT: 6 sins first (critical for geometry), then signs/abs
            TRG = pool.tile([P, 4, FW], F16, tag="TRG")   # cp sp ct st
            DS = pool.tile([P, 2, FW], F16, tag="DS")     # cd sd (raw)
            nc.scalar.activation(DS[:, 1, :], DTH, AF.Sin)
            nc.scalar.activation(DS[:, 0, :], DTH, AF.Sin, bias=PIH[:, :])
            nc.scalar.activation(TRG[:, 3, :], inp(15), AF.Sin)
            nc.scalar.activation(TRG[:, 2, :], inp(15), AF.Sin, bias=PIH[:, :])
            nc.scalar.activation(TRG[:, 1, :], inp(6), AF.Sin)
            nc.scalar.activation(TRG[:, 0, :], inp(6), AF.Sin, bias=PIH[:, :])
            SGN = pool.tile([P, 2, FW], F16, tag="SGN")   # sgn(cd) sgn(sd)
            nc.scalar.activation(SGN, DS, AF.Sign, bias=TINY[:, :])
            ACD = pool.tile([P, 2, FW], F16, tag="ACD")   # |cd| |sd|
            nc.scalar.activation(ACD, DS, AF.Abs)
            A4 = pool.tile([P, 4, FW], F16, tag="A4")     # |cp| |sp| |ct| |st|
            nc.scalar.activation(A4, TRG, AF.Abs)

            # DVE head preps
            HV = pool.tile([P, 4, FW], F16, tag="HV")     # l w a b (half dims)
            nc.vector.tensor_scalar(out=_ap(HV, 1, [(-1, 2)]), in0=IN[:, 3:5, :],
                                    scalar1=0.5, scalar2=None, op0=OP.mult)
            nc.vector.tensor_scalar(out=_ap(HV, 3, [(-1, 2)]), in0=IN[:, 12:14, :],
                                    scalar1=0.5, scalar2=None, op0=OP.mult)
            NAB = pool.tile([P, 2, FW], F16, tag="NAB")   # -a -b
            nc.vector.tensor_scalar(out=NAB, in0=HV[:, 2:4, :],
                                    scalar1=-1.0, scalar2=None, op0=OP.mult)
            DXY = pool.tile([P, 2, FW], F16, tag="DXY")   # dx dy (world)
            nc.vector.tensor_tensor(out=DXY, in0=IN[:, 0:2, :], in1=IN[:, 9:11, :],
                                    op=OP.subtract)
            AD2 = pool.tile([P, 2, FW], F16, tag="AD2")   # |dx| |dy|
            nc.scalar.activation(AD2, DXY, AF.Abs)

            # --- SL1 preps early (ACT consumes during geometry) -----------
            D4 = pool.tile([P, 4, FW], F16, tag="T8")     # dz dh dvx dvy
            nc.vector.tensor_tensor(out=D4[:, 0, :], in0=inp(2), in1=inp(11), op=OP.subtract)
            nc.vector.tensor_tensor(out=D4[:, 1, :], in0=inp(5), in1=inp(14), op=OP.subtract)
            nc.vector.tensor_tensor(out=D4[:, 2:4, :], in0=IN[:, 7:9, :],
                                    in1=IN[:, 16:18, :], op=OP.subtract)
            WD4 = pool.tile([P, 4, FW], F16, tag="WD4")
            nc.vector.tensor_tensor(out=WD4, in0=D4, in1=bc(IN, 21, 4), op=OP.mult)
            RL4 = pool.tile([P, 4, FW], F16, tag="RL4")   # relu(|wd|-1)
            nc.scalar.activation(RL4, WD4, AF.Abs)
            nc.scalar.activation(RL4, RL4, AF.Relu, bias=NEG1[:, :])
            JKA = pool.tile([P, 2, FW], F16, tag="JKA")   # ACT junk sink
            nc.scalar.activation(JKA[:, 0, :], WD4[:, 0, :], AF.Square,
                                 accum_out=ACC[:, A_ZSQ:A_ZSQ + 1])
            nc.scalar.activation(JKA[:, 0, :], RL4[:, 0, :], AF.Square,
                                 accum_out=ACC[:, A_ZRL:A_ZRL + 1])
            nc.scalar.activation(JKA[:, 0, :], WD4[:, 1, :], AF.Square,
                                 accum_out=ACC[:, A_HSQ:A_HSQ + 1])
            nc.scalar.activation(JKA[:, 0, :], RL4[:, 1, :], AF.Square,
                                 accum_out=ACC[:, A_HRL:A_HRL + 1])
            nc.scalar.activation(JKA, WD4[:, 2:4, :], AF.Square,
                                 accum_out=ACC[:, A_VSQ:A_VSQ + 1])
            nc.scalar.activation(JKA, RL4[:, 2:4, :], AF.Square,
                                 accum_out=ACC[:, A_VRL:A_VRL + 1])
            nc.scalar.activation(JKA[:, 0, :], inp(21), AF.Copy,
                                 accum_out=ACC[:, A_W:A_W + 1])

            # --- enclosing-box head: launch Pool chain early --------------
            # E8 = (l|cp|, l|sp|, w|cp|, w|sp|, a|ct|, a|st|, b|ct|, b|st|)
            E8 = pool.tile([P, 8, FW], F16, tag="E8")
            nc.vector.tensor_tensor(out=E8[:, 0:4, :],
                                    in0=_ap(HV, 0, [(1, 2), (0, 2)]),
                                    in1=_ap(A4, 0, [(0, 2), (1, 2)]), op=OP.mult)
            nc.vector.tensor_tensor(out=E8[:, 4:8, :],
                                    in0=_ap(HV, 2, [(1, 2), (0, 2)]),
                                    in1=_ap(A4, 2, [(0, 2), (1, 2)]), op=OP.mult)
            # EXY = (ex_p, ey_p, ex_t, ey_t): ex = l|c|+w|s| ; ey = l|s|+w|c|
            EXY = pool.tile([P, 4, FW], F16, tag="EXY")
            nc.vector.tensor_tensor(out=EXY, in0=_ap(E8, 0, [(4, 2), (1, 2)]),
                                    in1=_ap(E8, 3, [(4, 2), (-1, 2)]), op=OP.add)
            DEL = pool.tile([P, 2, FW], F16, tag="DEL")
            SUM = pool.tile([P, 2, FW], F16, tag="SUM")
            nc.gpsimd.tensor_tensor(out=DEL, in0=EXY[:, 0:2, :], in1=EXY[:, 2:4, :],
                                    op=OP.subtract)
            nc.gpsimd.tensor_tensor(out=SUM, in0=EXY[:, 0:2, :], in1=EXY[:, 2:4, :],
                                    op=OP.add)
            ADL = pool.tile([P, 2, FW], F16, tag="ADL")
            nc.scalar.activation(ADL, DEL, AF.Abs)
            MXD = pool.tile([P, 2, FW], F16, tag="DEL")   # reuse DEL
            nc.vector.tensor_tensor(out=MXD, in0=ADL, in1=AD2, op=OP.max)
            W2 = pool.tile([P, 2, FW], F16, tag="ADL")    # reuse ADL
            nc.gpsimd.tensor_tensor(out=W2, in0=SUM, in1=MXD, op=OP.add)
            WSQ = pool.tile([P, 2, FW], F32, tag="F32A")
            nc.gpsimd.tensor_tensor(out=WSQ, in0=W2, in1=W2, op=OP.mult)
            C2V = pool.tile([P, FW], F32, tag="C2V")
            nc.gpsimd.tensor_tensor(out=C2V, in0=WSQ[:, 0, :], in1=WSQ[:, 1, :], op=OP.add)

            # --- BCE (ACT + small DVE) ------------------------------------
            SP = pool.tile([P, FW], F16, tag="SP")
            nc.scalar.activation(SP, inp(18), AF.Exp)
            nc.scalar.activation(SP, SP, AF.Ln, bias=1.0)
            WSP = pool.tile([P, FW], F16, tag="WSP")
            nc.vector.tensor_tensor(out=WSP, in0=SP, in1=inp(21), op=OP.mult)
            nc.scalar.activation(JKA[:, 0, :], WSP, AF.Copy,
                                 accum_out=ACC[:, A_WSP:A_WSP + 1])
            WIP = pool.tile([P, FW], F16, tag="WIP")
            nc.vector.tensor_tensor(out=WIP, in0=inp(18), in1=inp(21), op=OP.mult)
            WIT = pool.tile([P, FW], F16, tag="JKV")
            nc.vector.tensor_tensor(out=WIT, in0=WIP, in1=inp(19), op=OP.mult)
            nc.scalar.activation(JKA[:, 0, :], WIT, AF.Copy,
                                 accum_out=ACC[:, A_WIT:A_WIT + 1])

            # focal exps early on ACT (needs DMA piece 2)
            ET = pool.tile([P, 10, FW], F16, tag="ET")
            nc.scalar.activation(ET, IN[:, 22:32, :], AF.Exp)

            # ============== geometry: clamped trig ========================
            AC2 = pool.tile([P, 2, FW], F16, tag="AC2")   # |cd|' |sd|' clamped
            nc.vector.tensor_scalar(out=AC2, in0=ACD, scalar1=EPSC, scalar2=None, op0=OP.max)
            CS2 = pool.tile([P, 2, FW], F16, tag="CS2")   # c~ s~
            nc.vector.tensor_tensor(out=CS2, in0=SGN, in1=AC2, op=OP.mult)
            AC32 = pool.tile([P, 2, FW], F32, tag="F32B")
            nc.vector.tensor_copy(out=AC32, in_=AC2)
            RAC = pool.tile([P, 2, FW], F32, tag="F32C")  # 1/|c|' 1/|s|'
            nc.vector.reciprocal_approx_fast(out=RAC.rearrange("p a b -> p (a b)"),
                                             in_=AC32.rearrange("p a b -> p (a b)"))
            Q2 = pool.tile([P, 2, FW], F16, tag="ACD")    # c*ss, s*sc (reuse ACD)
            nc.vector.tensor_tensor(out=Q2, in0=CS2, in1=_ap(SGN, 1, [(-1, 2)]), op=OP.mult)
            RHO = pool.tile([P, 2, FW], F16, tag="RHO")   # c/s, s/c (signed)
            nc.vector.tensor_tensor(out=RHO, in0=Q2, in1=_ap(RAC, 1, [(-1, 2)]), op=OP.mult)
            RP2 = pool.tile([P, 2, FW], F16, tag="RP2")   # (c/s, -s/c)
            nc.vector.tensor_copy(out=RP2[:, 0, :], in_=RHO[:, 0, :])
            nc.vector.tensor_scalar(out=RP2[:, 1, :], in0=RHO[:, 1, :],
                                    scalar1=-1.0, scalar2=None, op0=OP.mult)
            MU2 = pool.tile([P, 2, FW], F16, tag="DS")    # |s|/|c|, |c|/|s| (reuse DS)
            nc.vector.tensor_tensor(out=MU2, in0=_ap(AC2, 1, [(-1, 2)]), in1=RAC, op=OP.mult)
            MU2S = pool.tile([P, 2, FW], F16, tag="RHO")  # signed (reuse RHO)
            nc.vector.tensor_tensor(out=MU2S, in0=MU2, in1=SGN, op=OP.mult)
            MU4 = pool.tile([P, 4, FW], F16, tag="MU4")   # -m1 -m2 m1 m2
            nc.vector.tensor_copy(out=MU4[:, 2:4, :], in_=MU2S)
            nc.vector.tensor_scalar(out=MU4[:, 0:2, :], in0=MU2S,
                                    scalar1=-1.0, scalar2=None, op0=OP.mult)
            HMU4 = pool.tile([P, 4, FW], F16, tag="HMU4")
            nc.vector.tensor_scalar(out=HMU4, in0=MU4, scalar1=0.5, scalar2=None, op0=OP.mult)

            # A center in B frame: X = ct*dx+st*dy ; Y = ct*dy-st*dx
            RP5 = pool.tile([P, 5, FW], F16, tag="RP5")
            nc.vector.tensor_tensor(out=RP5[:, 0:4, :],
                                    in0=_ap(DXY, 0, [(0, 2), (1, 2)]),
                                    in1=_ap(TRG, 2, [(1, 2), (0, 2)]), op=OP.mult)
            nc.vector.tensor_scalar(out=RP5[:, 4, :], in0=RP5[:, 2, :],
                                    scalar1=-1.0, scalar2=None, op0=OP.mult)
            XY = pool.tile([P, 2, FW], F16, tag="XY")
            nc.vector.tensor_tensor(out=XY, in0=RP5[:, 0:2, :],
                                    in1=_ap(RP5, 3, [(1, 2)]), op=OP.add)

            # corner offsets: T8 = (lc, ls, wc, ws, -lc, -ls, -wc, -ws)
            T8 = pool.tile([P, 8, FW], F16, tag="T8")     # reuse D4 buffer
            nc.vector.tensor_tensor(out=T8[:, 0:4, :],
                                    in0=_ap(HV, 0, [(1, 2), (0, 2)]),
                                    in1=_ap(CS2, 0, [(0, 2), (1, 2)]), op=OP.mult)
            nc.vector.tensor_scalar(out=T8[:, 4:8, :], in0=T8[:, 0:4, :],
                                    scalar1=-1.0, scalar2=None, op0=OP.mult)
            OFX = pool.tile([P, 4, FW], F16, tag="OFX")
            nc.vector.tensor_tensor(out=OFX[:, 0:2, :], in0=_ap(T8, 0, [(4, 2)]),
                                    in1=_ap(T8, 3, [(0, 2)]), op=OP.add)
            nc.vector.tensor_scalar(out=OFX[:, 2:4, :], in0=OFX[:, 0:2, :],
                                    scalar1=-1.0, scalar2=None, op0=OP.mult)
            OFY = pool.tile([P, 4, FW], F16, tag="OFY")
            nc.vector.tensor_tensor(out=OFY[:, 0:2, :], in0=_ap(T8, 1, [(4, 2)]),
                                    in1=_ap(T8, 6, [(0, 2)]), op=OP.add)
            nc.vector.tensor_scalar(out=OFY[:, 2:4, :], in0=OFY[:, 0:2, :],
                                    scalar1=-1.0, scalar2=None, op0=OP.mult)
            CX = pool.tile([P, 4, FW], F16, tag="CX")
            nc.vector.tensor_tensor(out=CX, in0=bc(XY, 0, 4), in1=OFX, op=OP.add)
            CY = pool.tile([P, 5, FW], F16, tag="RP5")    # reuse RP5
            nc.vector.tensor_tensor(out=CY[:, 0:4, :], in0=bc(XY, 1, 4), in1=OFY, op=OP.add)
            nc.vector.tensor_copy(out=CY[:, 4, :], in_=CY[:, 0, :])

            # y-interval clip per edge (YL/YH reuse OFX/OFY buffers)
            YL = pool.tile([P, 4, FW], F16, tag="OFX")
            YH = pool.tile([P, 4, FW], F16, tag="OFY")
            nc.vector.tensor_tensor(out=YL, in0=CY[:, 0:4, :], in1=CY[:, 1:5, :], op=OP.min)
            nc.vector.tensor_tensor(out=YH, in0=CY[:, 0:4, :], in1=CY[:, 1:5, :], op=OP.max)
            nc.vector.tensor_tensor(out=YL, in0=YL, in1=bc(NAB, 1, 4), op=OP.max)
            nc.vector.tensor_tensor(out=YH, in0=YH, in1=bc(HV, 3, 4), op=OP.min)
            nc.vector.tensor_tensor(out=YH, in0=YH, in1=YL, op=OP.max)

            T1 = pool.tile([P, 4, FW], F16, tag="T1")
            T2 = pool.tile([P, 4, FW], F16, tag="T2")
            nc.vector.tensor_tensor(out=T1, in0=YL, in1=CY[:, 0:4, :], op=OP.subtract)
            nc.vector.tensor_tensor(out=T2, in0=YH, in1=CY[:, 0:4, :], op=OP.subtract)
            rp_pat = _ap(RP2, 0, [(0, 2), (1, 2)])
            XLO = pool.tile([P, 4, FW], F16, tag="XLO")
            XHI = pool.tile([P, 4, FW], F16, tag="XHI")
            nc.vector.tensor_tensor(out=XLO, in0=T1, in1=rp_pat, op=OP.mult)
            nc.vector.tensor_tensor(out=XLO, in0=XLO, in1=CX, op=OP.add)
            nc.vector.tensor_tensor(out=XHI, in0=T2, in1=rp_pat, op=OP.mult)
            nc.vector.tensor_tensor(out=XHI, in0=XHI, in1=CX, op=OP.add)

            CA = pool.tile([P, 4, FW], F16, tag="CA")
            CB = pool.tile([P, 4, FW], F16, tag="CB")
            nc.vector.tensor_tensor(out=CA, in0=XLO, in1=bc(HV, 2, 4), op=OP.min)
            nc.vector.tensor_tensor(out=CA, in0=CA, in1=bc(NAB, 0, 4), op=OP.max)
            nc.vector.tensor_tensor(out=CB, in0=XHI, in1=bc(HV, 2, 4), op=OP.min)
            nc.vector.tensor_tensor(out=CB, in0=CB, in1=bc(NAB, 0, 4), op=OP.max)

            # Phi diff: (cb*xhi - ca*xlo) - 0.5*(cb-ca)*(cb+ca)
            PA = pool.tile([P, 4, FW], F16, tag="T1")     # reuse T1
            PB = pool.tile([P, 4, FW], F16, tag="T2")     # reuse T2
            nc.vector.tensor_tensor(out=PA, in0=CA, in1=XLO, op=OP.mult)
            nc.vector.tensor_tensor(out=PB, in0=CB, in1=XHI, op=OP.mult)
            D1 = pool.tile([P, 4, FW], F16, tag="OFX")    # reuse (YL dead)
            nc.vector.tensor_tensor(out=D1, in0=PB, in1=PA, op=OP.subtract)
            DM = pool.tile([P, 4, FW], F16, tag="XLO")    # reuse XLO
            DP = pool.tile([P, 4, FW], F16, tag="XHI")    # reuse XHI
            nc.vector.tensor_tensor(out=DM, in0=CB, in1=CA, op=OP.subtract)
            nc.vector.tensor_tensor(out=DP, in0=CB, in1=CA, op=OP.add)
            DMDP = pool.tile([P, 4, FW], F16, tag="CA")   # reuse CA
            nc.vector.tensor_tensor(out=DMDP, in0=DM, in1=DP, op=OP.mult)
            C1 = pool.tile([P, 4, FW], F16, tag="CB")     # reuse CB
            nc.vector.tensor_tensor(out=C1, in0=MU4, in1=D1, op=OP.mult)
            C2T = pool.tile([P, 4, FW], F16, tag="T1")    # reuse (PA dead)
            nc.vector.tensor_tensor(out=C2T, in0=HMU4, in1=DMDP, op=OP.mult)
            CT4 = pool.tile([P, 4, FW], F16, tag="T2")    # reuse (PB dead)
            nc.vector.tensor_tensor(out=CT4, in0=C1, in1=C2T, op=OP.subtract)
            R2 = pool.tile([P, 2, FW], F16, tag="XY")     # reuse XY
            nc.vector.tensor_tensor(out=R2, in0=CT4[:, 0:2, :], in1=CT4[:, 2:4, :], op=OP.add)
            R1 = pool.tile([P, FW], F16, tag="DTH")       # reuse DTH
            nc.vector.tensor_tensor(out=R1, in0=R2[:, 0, :], in1=R2[:, 1, :], op=OP.add)
            INTER = pool.tile([P, FW], F16, tag="INTER")
            nc.scalar.activation(INTER, R1, AF.Abs)

            # iou = inter / max(4(lw+ab) - inter, 1e-7), clamped to [0,1]
            UAB = pool.tile([P, 2, FW], F16, tag="AC2")   # reuse AC2
            nc.vector.tensor_tensor(out=UAB, in0=_ap(HV, 0, [(2, 2)]),
                                    in1=_ap(HV, 1, [(2, 2)]), op=OP.mult)
            US = pool.tile([P, FW], F16, tag="US")
            nc.vector.tensor_tensor(out=US, in0=UAB[:, 0, :], in1=UAB[:, 1, :], op=OP.add)
            U4 = pool.tile([P, FW], F16, tag="U4")
            nc.vector.tensor_scalar(out=U4, in0=US, scalar1=4.0, scalar2=None, op0=OP.mult)
            nc.vector.tensor_tensor(out=U4, in0=U4, in1=INTER, op=OP.subtract)
            UG = pool.tile([P, FW], F32, tag="F32B")      # reuse AC32
            nc.vector.tensor_scalar(out=UG, in0=U4, scalar1=1e-7, scalar2=None, op0=OP.max)
            RU = pool.tile([P, FW], F32, tag="F32C")      # reuse RAC
            nc.vector.reciprocal_approx_fast(out=RU, in_=UG)
            IOU = pool.tile([P, FW], F16, tag="IOU")
            nc.vector.tensor_tensor(out=IOU, in0=INTER, in1=RU, op=OP.mult)
            nc.vector.tensor_scalar(out=IOU, in0=IOU, scalar1=1.0, scalar2=None, op0=OP.min)

            # enclosing tail (f32)
            nc.vector.tensor_scalar(out=C2V, in0=C2V, scalar1=1e-7, scalar2=None, op0=OP.max)
            RC2 = pool.tile([P, FW], F32, tag="RC2")
            nc.vector.reciprocal_approx_fast(out=RC2, in_=C2V)
            DD2 = pool.tile([P, 2, FW], F16, tag="SUM")   # reuse SUM
            nc.vector.tensor_tensor(out=DD2, in0=DXY, in1=DXY, op=OP.mult)
            D2 = pool.tile([P, FW], F32, tag="F32A")      # reuse WSQ
            nc.vector.tensor_tensor(out=D2, in0=DD2[:, 0, :], in1=DD2[:, 1, :], op=OP.add)
            DL = pool.tile([P, FW], F32, tag="DL")
            nc.vector.tensor_tensor(out=DL, in0=D2, in1=RC2, op=OP.mult)
            DLM = pool.tile([P, FW], F16, tag="US")       # reuse US
            nc.vector.tensor_tensor(out=DLM, in0=DL, in1=IOU, op=OP.subtract)
            WDL = pool.tile([P, FW], F16, tag="JKV")
            nc.vector.tensor_tensor(out=WDL, in0=DLM, in1=inp(21), op=OP.mult)
            nc.scalar.activation(JKA[:, 0, :], WDL, AF.Copy,
                                 accum_out=ACC[:, A_DIOU:A_DIOU + 1])

            # ===================== focal =================================
            S5 = pool.tile([P, 5, FW], F16, tag="S5")
            nc.vector.tensor_tensor(out=S5, in0=ET[:, 0:5, :], in1=ET[:, 5:10, :], op=OP.add)
            S22 = pool.tile([P, 2, FW], F16, tag="S22")
            nc.vector.tensor_tensor(out=S22, in0=S5[:, 0:2, :], in1=S5[:, 2:4, :], op=OP.add)
            SS = pool.tile([P, FW], F16, tag="SS")
            nc.vector.tensor_tensor(out=SS, in0=S22[:, 0, :], in1=S22[:, 1, :], op=OP.add)
            nc.vector.tensor_tensor(out=SS, in0=SS, in1=S5[:, 4, :], op=OP.add)

            MT = pool.tile([P, 10, FW], F16, tag="ET")    # reuse ET (dead after S5)
            for c in range(10):
                nc.vector.tensor_scalar(out=MT[:, c, :], in0=inp(20), scalar1=float(c),
                                        scalar2=None, op0=OP.is_equal)
            nc.vector.tensor_tensor(out=MT, in0=MT, in1=IN[:, 22:32, :], op=OP.mult)
            L5 = pool.tile([P, 5, FW], F16, tag="S5")     # reuse S5 (dead)
            nc.vector.tensor_tensor(out=L5, in0=MT[:, 0:5, :], in1=MT[:, 5:10, :], op=OP.add)
            L22 = pool.tile([P, 2, FW], F16, tag="S22")
            nc.vector.tensor_tensor(out=L22, in0=L5[:, 0:2, :], in1=L5[:, 2:4, :], op=OP.add)
            LT = pool.tile([P, FW], F16, tag="LT")
            nc.vector.tensor_tensor(out=LT, in0=L22[:, 0, :], in1=L22[:, 1, :], op=OP.add)
            nc.vector.tensor_tensor(out=LT, in0=LT, in1=L5[:, 4, :], op=OP.add)

            LNS = pool.tile([P, FW], F16, tag="U4")       # reuse U4
            nc.scalar.activation(LNS, SS, AF.Ln)
            LPT = pool.tile([P, FW], F16, tag="LPT")
            nc.vector.tensor_tensor(out=LPT, in0=LT, in1=LNS, op=OP.subtract)
            PTT = pool.tile([P, FW], F16, tag="SS")       # reuse SS
            nc.scalar.activation(PTT, LPT, AF.Exp)
            OM = pool.tile([P, FW], F16, tag="LT")        # reuse LT
            nc.vector.tensor_scalar(out=OM, in0=PTT, scalar1=-1.0, scalar2=1.0,
                                    op0=OP.mult, op1=OP.add)
            MP = pool.tile([P, FW], F16, tag="MP")
            nc.vector.tensor_scalar(out=MP, in0=inp(20), scalar1=0.5, scalar2=None, op0=OP.is_gt)
            nc.vector.tensor_scalar(out=MP, in0=MP, scalar1=-0.5, scalar2=0.75,
                                    op0=OP.mult, op1=OP.add)
            F1 = pool.tile([P, FW], F16, tag="INTER")     # reuse INTER
            nc.vector.tensor_tensor(out=F1, in0=OM, in1=LPT, op=OP.mult)
            nc.vector.tensor_tensor(out=F1, in0=F1, in1=MP, op=OP.mult)
            F2 = pool.tile([P, FW], F16, tag="JKV")
            nc.vector.tensor_tensor(out=F2, in0=F1, in1=OM, op=OP.mult)
            nc.scalar.activation(JKA[:, 0, :], F2, AF.Copy, scale=-1.0,
                                 accum_out=ACC[:, A_FOC:A_FOC + 1])

            # ---------- cross-partition reduce + output ----------
            PS = ppool.tile([1, 32], F32)
            nc.tensor.matmul(PS, ones, ACC, start=True, stop=True)
            OUT = spool.tile([1, 32], F32)
            nc.scalar.copy(out=OUT, in_=PS)
            nc.sync.dma_start(out=outp[:, :], in_=OUT)
    nc.compile()
    return nc


_NC_CACHE = None


def _get_nc():
    global _NC_CACHE
    if _NC_CACHE is None:
        _NC_CACHE = build_bass()
    return _NC_CACHE


def pack_inputs(cls_pred, reg_pred, iou_pred, reg_targets, iou_targets,
                cls_targets, reg_weights):
    """Returns list of 8 per-core input dicts."""
    B = cls_pred.shape[0]
    maps = []
    for b in range(B):
        h = np.empty((NSLOT, P, FW), np.float16)
        h[0:9] = np.asarray(reg_pred[b], np.float32).reshape(9, P, FW)
        h[9:18] = np.asarray(reg_targets[b], np.float32).reshape(9, P, FW)
        h[18] = np.asarray(iou_pred[b], np.float32).reshape(P, FW)
        h[19] = np.asarray(iou_targets[b], np.float32).reshape(P, FW)
        h[20] = np.asarray(cls_targets[b]).astype(np.float32).reshape(P, FW)
        h[21] = np.asarray(reg_weights[b]).astype(np.float32).reshape(P, FW)
        h[22:32] = np.asarray(cls_pred[b], np.float32).reshape(10, P, FW)
        maps.append({"h16": np.ascontiguousarray(h.transpose(1, 0, 2))})
    return maps


def partials_from_acc(acc):
    """acc: raw [1,32] per-core sums -> golden-style 9 partials
    [focal_s, valid_s, diou_s, z_s, h_s, vx_s, vy_s(joint half), bce_s, w_s]
    (vx/vy split is not recoverable; both halves of vel reported evenly)."""
    a = np.asarray(acc, np.float64).reshape(32)
    w_s = a[A_W]
    focal_s = a[A_FOC]
    diou_s = a[A_DIOU]
    z_s = 0.5 * (a[A_ZSQ] - a[A_ZRL]) + 0.5 * w_s
    h_s = 0.5 * (a[A_HSQ] - a[A_HRL]) + 0.5 * w_s
    vel_s = 0.5 * (a[A_VSQ] - a[A_VRL]) + w_s          # vx_s + vy_s
    bce_s = a[A_WSP] - a[A_WIT]
    return np.array([focal_s, 65536.0, diou_s, z_s, h_s, 0.5 * vel_s,
                     0.5 * vel_s, bce_s, w_s])


def combine(parts):
    """parts: [8, 1, 32] per-core raw sums -> final [7] float32."""
    a = np.asarray(parts, np.float64).sum(0).reshape(32)
    w_s = max(a[A_W], 1.0)
    n_valid = 8.0 * 65536.0
    cls_loss = a[A_FOC] / n_valid
    bev_loss = (a[A_DIOU] + a[A_W]) / w_s
    z_loss = 0.5 * (a[A_ZSQ] - a[A_ZRL]) / w_s
    h_loss = 0.5 * (a[A_HSQ] - a[A_HRL]) / w_s
    vel_loss = 0.5 * (a[A_VSQ] - a[A_VRL]) / w_s
    iou_loss = (a[A_WSP] - a[A_WIT]) / w_s
    total = cls_loss + 2.0 * bev_loss + z_loss + h_loss + vel_loss + iou_loss
    return np.array([total, cls_loss, bev_loss, z_loss, h_loss, vel_loss, iou_loss],
                    np.float32)


def kernel(cls_pred, reg_pred, iou_pred, reg_targets, iou_targets,
           cls_targets, reg_weights, _trace=False):
    cls_pred, reg_pred, iou_pred, reg_targets, iou_targets, cls_targets, reg_weights = (
        np.asarray(a) for a in (cls_pred, reg_pred, iou_pred, reg_targets,
                                iou_targets, cls_targets, reg_weights))
    nc = _get_nc()
    in_maps = pack_inputs(cls_pred, reg_pred, iou_pred, reg_targets,
                          iou_targets, cls_targets, reg_weights)
    res = run_bass_kernel_spmd(nc, in_maps, core_ids=list(range(8)), trace=_trace)
    parts = [res.results[i]["out"] for i in range(8)]
    out = combine(parts)
    if _trace:
        return out, res
    return out


# revision 9
# speedup vs baseline: 1.5485x; 1.0102x over previous
"""DetectionBEVLoss Trainium2 kernel: 8-core data-parallel (1 batch/core).

Layout: per core 65536 sites as [128 partitions, 512 free]. Host packs all
inputs into one fp16 array [128, 32, 512] per core (slot map below), with
yaws first so trig starts as soon as the first small DMA piece lands.

Rotated intersection via a single-direction Green's-theorem identity:
  area(A ∩ B) = sum over A's 4 edges of  int clamp_a(x(y)) dy
with the y-integral restricted to |y| <= b (B = target box, axis-aligned
in its own frame, half-dims a x b; A = pred box with corners in B's
frame). Each edge needs one y-interval clip and a clamp antiderivative
Phi(x) = c*x - c^2/2 at the two clipped endpoints; the only reciprocals
are 1/|cos dth| and 1/|sin dth| (clamped at 2^-6, with the perturbation
applied consistently to the corner geometry).

Trig runs on the Scalar (ACT) engine via the sin table (cos = sin(x+pi/2)).
Smooth-L1 uses sl1(d)+0.5 = 0.5 d^2 + 0.5 - 0.5 relu(|d|-1)^2 so the sums
are plain Square-accumulates on ACT. BCE uses softplus(x) - x*t. The Pool
engine carries the enclosing-box chain, union/areas, d^2, and the softmax
sum tree.
"""
import math

import numpy as np

import concourse.bacc as bacc
import concourse.bass as bass
import concourse.mybir as mybir
import concourse.tile as tile
from concourse.bass_utils import run_bass_kernel_spmd

F16 = mybir.dt.float16
F32 = mybir.dt.float32
OP = mybir.AluOpType
AF = mybir.ActivationFunctionType

P = 128          # partitions
FW = 512         # free width per partition (128*512 = 65536 sites/core)

# slot map in the packed fp16 input [128, 32, 512]
# 0: yawp, 1: yawt | 2: xp, 3: yp, 4: xt, 5: yt | 6: w3p, 7: l4p, 8: w3t,
# 9: l4t | 10: zp, 11: hp, 12: vxp, 13: vyp | 14: zt, 15: ht, 16: vxt,
# 17: vyt | 18: iou_pred, 19: iou_targets, 20: cls_t, 21: w | 22-31: cls_pred
NSLOT = 32

EPSC = 2.0 ** -6     # |cos|,|sin| clamp (slope guard)

# ACC slots (fp32 [P,32])
A_FOC, A_DIOU, A_ZSQ, A_ZRL, A_HSQ, A_HRL, A_VSQ, A_VRL, A_WSP, A_WIT, A_W = range(11)


def _ap(t, s0, slot_dims, col0=0, ncol=FW, colstep=1):
    """Manual AP into tile t ([128, S, W]): base slot s0, then
    (slot_step, count) dims, innermost column dim."""
    ss = t.ap[-2][0]
    ap = [list(t.ap[0])] + [[s * ss, c] for s, c in slot_dims] + [[colstep, ncol]]
    return bass.AP(tensor=t.tensor, offset=t.offset + s0 * ss + col0, ap=ap)


def build_bass():
    nc = bacc.Bacc("TRN2", target_bir_lowering=False, debug=False)
    h16 = nc.declare_dram_parameter("h16", [P, NSLOT, FW], F16, isOutput=False)
    outp = nc.declare_dram_parameter("out", [1, 32], F32, isOutput=True)

    with tile.TileContext(nc) as tc:
        with (
            tc.tile_pool(name="main", bufs=1) as pool,
            tc.tile_pool(name="small", bufs=1) as spool,
            tc.tile_pool(name="ps", bufs=1, space="PSUM") as ppool,
        ):
            IN = pool.tile([P, NSLOT, FW], F16)
            nc.sync.dma_start(out=IN[:, 0:2, :], in_=h16[:, 0:2, :])
            nc.sync.dma_start(out=IN[:, 2:10, :], in_=h16[:, 2:10, :])
            nc.sync.dma_start(out=IN[:, 10:22, :], in_=h16[:, 10:22, :])
            nc.sync.dma_start(out=IN[:, 22:32, :], in_=h16[:, 22:32, :])

            ones = spool.tile([P, 1], F32)
            nc.vector.memset(ones, 1.0)
            ACC = spool.tile([P, 32], F32)
            nc.vector.memset(ACC, 0.0)
            PIH = spool.tile([P, 1], F32)
            nc.vector.memset(PIH, math.pi / 2)
            NEG1 = spool.tile([P, 1], F32)
            nc.vector.memset(NEG1, -1.0)
            TINY = spool.tile([P, 1], F32)
            nc.vector.memset(TINY, 1e-6)

            def inp(s):
                return IN[:, s, :]

            def bc(t, s, n):  # broadcast slot s of tile t over n slots
                return _ap(t, s, [(0, n)])

            # ================= head ======================================
            DTH = pool.tile([P, FW], F16, tag="DTH")
            nc.vector.tensor_tensor(out=DTH, in0=inp(0), in1=inp(1), op=OP.subtract)

            # TRG slots: 0:cp 1:sp 2:ct 3:st 4:cd 5:sd
            TRG = pool.tile([P, 6, FW], F16, tag="TRG")
            nc.scalar.activation(TRG[:, 5, :], DTH, AF.Sin)
            nc.scalar.activation(TRG[:, 4, :], DTH, AF.Sin, bias=PIH[:, :])
            SGN4 = pool.tile([P, 4, FW], F16, tag="SGN4")  # scd ssd -scd -ssd
            nc.scalar.activation(SGN4[:, 0:2, :], TRG[:, 4:6, :], AF.Sign, bias=TINY[:, :])
            nc.scalar.activation(TRG[:, 3, :], inp(1), AF.Sin)
            nc.scalar.activation(TRG[:, 2, :], inp(1), AF.Sin, bias=PIH[:, :])
            nc.scalar.activation(TRG[:, 1, :], inp(0), AF.Sin)
            nc.scalar.activation(TRG[:, 0, :], inp(0), AF.Sin, bias=PIH[:, :])
            A6 = pool.tile([P, 6, FW], F16, tag="A6")  # |cp| |sp| |ct| |st| |cd| |sd|
            nc.scalar.activation(A6, TRG, AF.Abs)

            # DVE head preps (only need DMA pieces 1-2)
            HV = pool.tile([P, 4, FW], F16, tag="HV")     # l w a b (half dims)
            nc.vector.tensor_scalar(out=_ap(HV, 1, [(-1, 2)]), in0=IN[:, 6:8, :],
                                    scalar1=0.5, scalar2=None, op0=OP.mult)
            nc.vector.tensor_scalar(out=_ap(HV, 3, [(-1, 2)]), in0=IN[:, 8:10, :],
                                    scalar1=0.5, scalar2=None, op0=OP.mult)
            NAB = pool.tile([P, 2, FW], F16, tag="NAB")   # -a -b
            nc.vector.tensor_scalar(out=NAB, in0=HV[:, 2:4, :],
                                    scalar1=-1.0, scalar2=None, op0=OP.mult)
            DXY = pool.tile([P, 2, FW], F16, tag="DXY")   # dx dy (world)
            nc.vector.tensor_tensor(out=DXY, in0=IN[:, 2:4, :], in1=IN[:, 4:6, :],
                                    op=OP.subtract)
            AD2 = pool.tile([P, 2, FW], F16, tag="AD2")   # |dx| |dy|
            nc.scalar.activation(AD2, DXY, AF.Abs)

            # --- SL1 preps early (ACT consumes during geometry) -----------
            D4 = pool.tile([P, 4, FW], F16, tag="T8")     # dz dh dvx dvy
            nc.vector.tensor_tensor(out=D4, in0=IN[:, 10:14, :], in1=IN[:, 14:18, :],
                                    op=OP.subtract)
            WD4 = pool.tile([P, 4, FW], F16, tag="WD4")
            nc.vector.tensor_tensor(out=WD4, in0=D4, in1=bc(IN, 21, 4), op=OP.mult)
            RL4 = pool.tile([P, 4, FW], F16, tag="RL4")   # relu(|wd|-1)
            nc.scalar.activation(RL4, WD4, AF.Abs)
            nc.scalar.activation(RL4, RL4, AF.Relu, bias=NEG1[:, :])
            JKA = pool.tile([P, 2, FW], F16, tag="JKA")   # ACT junk sink
            nc.scalar.activation(JKA[:, 0, :], WD4[:, 0, :], AF.Square,
                                 accum_out=ACC[:, A_ZSQ:A_ZSQ + 1])
            nc.scalar.activation(JKA[:, 0, :], RL4[:, 0, :], AF.Square,
                                 accum_out=ACC[:, A_ZRL:A_ZRL + 1])
            nc.scalar.activation(JKA[:, 0, :], WD4[:, 1, :], AF.Square,
                                 accum_out=ACC[:, A_HSQ:A_HSQ + 1])
            nc.scalar.activation(JKA[:, 0, :], RL4[:, 1, :], AF.Square,
                                 accum_out=ACC[:, A_HRL:A_HRL + 1])
            nc.scalar.activation(JKA, WD4[:, 2:4, :], AF.Square,
                                 accum_out=ACC[:, A_VSQ:A_VSQ + 1])
            nc.scalar.activation(JKA, RL4[:, 2:4, :], AF.Square,
                                 accum_out=ACC[:, A_VRL:A_VRL + 1])
            nc.scalar.activation(JKA[:, 0, :], inp(21), AF.Copy,
                                 accum_out=ACC[:, A_W:A_W + 1])

            # --- enclosing-box + union heads: feed the Pool engine --------
            # E8 = (l|cp|, l|sp|, w|cp|, w|sp|, a|ct|, a|st|, b|ct|, b|st|)
            E8 = pool.tile([P, 8, FW], F16, tag="E8")
            nc.vector.tensor_tensor(out=E8[:, 0:4, :],
                                    in0=_ap(HV, 0, [(1, 2), (0, 2)]),
                                    in1=_ap(A6, 0, [(0, 2), (1, 2)]), op=OP.mult)
            nc.vector.tensor_tensor(out=E8[:, 4:8, :],
                                    in0=_ap(HV, 2, [(1, 2), (0, 2)]),
                                    in1=_ap(A6, 2, [(0, 2), (1, 2)]), op=OP.mult)
            # EXY = (ex_p, ey_p, ex_t, ey_t): ex = l|c|+w|s| ; ey = l|s|+w|c|
            EXY = pool.tile([P, 4, FW], F16, tag="EXY")
            nc.vector.tensor_tensor(out=EXY, in0=_ap(E8, 0, [(4, 2), (1, 2)]),
                                    in1=_ap(E8, 3, [(4, 2), (-1, 2)]), op=OP.add)
            UAB = pool.tile([P, 2, FW], F16, tag="UAB")   # lw ab
            nc.gpsimd.tensor_tensor(out=UAB, in0=_ap(HV, 0, [(2, 2)]),
                                    in1=_ap(HV, 1, [(2, 2)]), op=OP.mult)
            US = pool.tile([P, FW], F16, tag="US")        # lw+ab
            nc.gpsimd.tensor_tensor(out=US, in0=UAB[:, 0, :], in1=UAB[:, 1, :], op=OP.add)
            DEL = pool.tile([P, 2, FW], F16, tag="DEL")
            SUM = pool.tile([P, 2, FW], F16, tag="SUM")
            nc.gpsimd.tensor_tensor(out=DEL, in0=EXY[:, 0:2, :], in1=EXY[:, 2:4, :],
                                    op=OP.subtract)
            nc.gpsimd.tensor_tensor(out=SUM, in0=EXY[:, 0:2, :], in1=EXY[:, 2:4, :],
                                    op=OP.add)
            ADL = pool.tile([P, 2, FW], F16, tag="ADL")
            nc.scalar.activation(ADL, DEL, AF.Abs)
            MXD = pool.tile([P, 2, FW], F16, tag="DEL")   # reuse DEL
            nc.vector.tensor_tensor(out=MXD, in0=ADL, in1=AD2, op=OP.max)
            W2 = pool.tile([P, 2, FW], F16, tag="ADL")    # reuse ADL
            nc.gpsimd.tensor_tensor(out=W2, in0=SUM, in1=MXD, op=OP.add)
            WSQ = pool.tile([P, 2, FW], F32, tag="F32A")
            nc.gpsimd.tensor_tensor(out=WSQ, in0=W2, in1=W2, op=OP.mult)
            C2V = pool.tile([P, FW], F32, tag="C2V")
            nc.gpsimd.tensor_tensor(out=C2V, in0=WSQ[:, 0, :], in1=WSQ[:, 1, :], op=OP.add)
            DD2 = pool.tile([P, 2, FW], F16, tag="SUM")   # reuse SUM
            nc.gpsimd.tensor_tensor(out=DD2, in0=DXY, in1=DXY, op=OP.mult)
            D2 = pool.tile([P, FW], F32, tag="D2")
            nc.gpsimd.tensor_tensor(out=D2, in0=DD2[:, 0, :], in1=DD2[:, 1, :], op=OP.add)

            # --- BCE (ACT + small DVE) ------------------------------------
            SP = pool.tile([P, FW], F16, tag="SP")
            nc.scalar.activation(SP, inp(18), AF.Exp)
            nc.scalar.activation(SP, SP, AF.Ln, bias=1.0)
            WSP = pool.tile([P, FW], F16, tag="WSP")
            nc.vector.tensor_tensor(out=WSP, in0=SP, in1=inp(21), op=OP.mult)
            nc.scalar.activation(JKA[:, 0, :], WSP, AF.Copy,
                                 accum_out=ACC[:, A_WSP:A_WSP + 1])
            WIP = pool.tile([P, FW], F16, tag="WIP")
            nc.vector.tensor_tensor(out=WIP, in0=inp(18), in1=inp(21), op=OP.mult)
            WIT = pool.tile([P, FW], F16, tag="JKV")
            nc.vector.tensor_tensor(out=WIT, in0=WIP, in1=inp(19), op=OP.mult)
            nc.scalar.activation(JKA[:, 0, :], WIT, AF.Copy,
                                 accum_out=ACC[:, A_WIT:A_WIT + 1])

            # focal exps early on ACT (needs DMA piece 4)
            ET = pool.tile([P, 10, FW], F16, tag="ET")
            nc.scalar.activation(ET, IN[:, 22:32, :], AF.Exp)
            # softmax-sum tree on Pool
            S5 = pool.tile([P, 5, FW], F16, tag="S5")
            nc.gpsimd.tensor_tensor(out=S5, in0=ET[:, 0:5, :], in1=ET[:, 5:10, :], op=OP.add)
            S22 = pool.tile([P, 2, FW], F16, tag="S22")
            nc.gpsimd.tensor_tensor(out=S22, in0=S5[:, 0:2, :], in1=S5[:, 2:4, :], op=OP.add)
            SS = pool.tile([P, FW], F16, tag="SS")
            nc.gpsimd.tensor_tensor(out=SS, in0=S22[:, 0, :], in1=S22[:, 1, :], op=OP.add)
            nc.gpsimd.tensor_tensor(out=SS, in0=SS, in1=S5[:, 4, :], op=OP.add)

            # ============== geometry: clamped trig ========================
            AC2 = pool.tile([P, 2, FW], F16, tag="AC2")   # |cd|' |sd|' clamped
            nc.vector.tensor_scalar(out=AC2, in0=A6[:, 4:6, :], scalar1=EPSC,
                                    scalar2=None, op0=OP.max)
            CS2 = pool.tile([P, 2, FW], F16, tag="CS2")   # c~ s~
            nc.vector.tensor_tensor(out=CS2, in0=SGN4[:, 0:2, :], in1=AC2, op=OP.mult)
            AC32 = pool.tile([P, 2, FW], F32, tag="F32B")
            nc.vector.tensor_copy(out=AC32, in_=AC2)
            RAC = pool.tile([P, 2, FW], F32, tag="F32C")  # 1/|c|' 1/|s|'
            nc.vector.reciprocal_approx_fast(out=RAC.rearrange("p a b -> p (a b)"),
                                             in_=AC32.rearrange("p a b -> p (a b)"))
            nc.vector.tensor_scalar(out=SGN4[:, 2:4, :], in0=SGN4[:, 0:2, :],
                                    scalar1=-1.0, scalar2=None, op0=OP.mult)
            Q2S = pool.tile([P, 2, FW], F16, tag="Q2S")   # (c*ss, -s*sc)
            nc.vector.tensor_tensor(out=Q2S, in0=CS2, in1=SGN4[:, 1:3, :], op=OP.mult)
            RP2 = pool.tile([P, 2, FW], F16, tag="RP2")   # (c/s, -s/c)
            nc.vector.tensor_tensor(out=RP2, in0=Q2S, in1=_ap(RAC, 1, [(-1, 2)]), op=OP.mult)
            MU2 = pool.tile([P, 2, FW], F16, tag="Q2S")   # |s|/|c|, |c|/|s| (reuse)
            nc.vector.tensor_tensor(out=MU2, in0=_ap(AC2, 1, [(-1, 2)]), in1=RAC, op=OP.mult)
            MU4 = pool.tile([P, 4, FW], F16, tag="MU4")   # -m1 -m2 m1 m2 (signed)
            nc.vector.tensor_tensor(out=MU4[:, 2:4, :], in0=MU2, in1=SGN4[:, 0:2, :], op=OP.mult)
            nc.vector.tensor_scalar(out=MU4[:, 0:2, :], in0=MU4[:, 2:4, :],
                                    scalar1=-1.0, scalar2=None, op0=OP.mult)
            HMU4 = pool.tile([P, 4, FW], F16, tag="HMU4")
            nc.vector.tensor_scalar(out=HMU4, in0=MU4, scalar1=0.5, scalar2=None, op0=OP.mult)

            # A center in B frame: X = ct*dx+st*dy ; Y = ct*dy-st*dx
            RP5 = pool.tile([P, 5, FW], F16, tag="RP5")
            nc.vector.tensor_tensor(out=RP5[:, 0:4, :],
                                    in0=_ap(DXY, 0, [(0, 2), (1, 2)]),
                                    in1=_ap(TRG, 2, [(1, 2), (0, 2)]), op=OP.mult)
            nc.vector.tensor_scalar(out=RP5[:, 4, :], in0=RP5[:, 2, :],
                                    scalar1=-1.0, scalar2=None, op0=OP.mult)
            XY = pool.tile([P, 2, FW], F16, tag="XY")
            nc.vector.tensor_tensor(out=XY, in0=RP5[:, 0:2, :],
                                    in1=_ap(RP5, 3, [(1, 2)]), op=OP.add)

            # corner offsets: T8 = (lc, ls, wc, ws, -lc, -ls, -wc, -ws)
            T8 = pool.tile([P, 8, FW], F16, tag="T8")     # reuse D4 buffer
            nc.vector.tensor_tensor(out=T8[:, 0:4, :],
                                    in0=_ap(HV, 0, [(1, 2), (0, 2)]),
                                    in1=_ap(CS2, 0, [(0, 2), (1, 2)]), op=OP.mult)
            nc.vector.tensor_scalar(out=T8[:, 4:8, :], in0=T8[:, 0:4, :],
                                    scalar1=-1.0, scalar2=None, op0=OP.mult)
            OFX = pool.tile([P, 4, FW], F16, tag="OFX")
            nc.vector.tensor_tensor(out=OFX[:, 0:2, :], in0=_ap(T8, 0, [(4, 2)]),
                                    in1=_ap(T8, 3, [(0, 2)]), op=OP.add)
            nc.vector.tensor_scalar(out=OFX[:, 2:4, :], in0=OFX[:, 0:2, :],
                                    scalar1=-1.0, scalar2=None, op0=OP.mult)
            OFY = pool.tile([P, 4, FW], F16, tag="OFY")
            nc.vector.tensor_tensor(out=OFY[:, 0:2, :], in0=_ap(T8, 1, [(4, 2)]),
                                    in1=_ap(T8, 6, [(0, 2)]), op=OP.add)
            nc.vector.tensor_scalar(out=OFY[:, 2:4, :], in0=OFY[:, 0:2, :],
                                    scalar1=-1.0, scalar2=None, op0=OP.mult)
            CX = pool.tile([P, 4, FW], F16, tag="CX")
            nc.vector.tensor_tensor(out=CX, in0=bc(XY, 0, 4), in1=OFX, op=OP.add)
            CY = pool.tile([P, 5, FW], F16, tag="RP5")    # reuse RP5
            nc.vector.tensor_tensor(out=CY[:, 0:4, :], in0=bc(XY, 1, 4), in1=OFY, op=OP.add)
            nc.vector.tensor_copy(out=CY[:, 4, :], in_=CY[:, 0, :])

            # y-interval clip per edge (YL/YH reuse OFX/OFY buffers)
            YL = pool.tile([P, 4, FW], F16, tag="OFX")
            YH = pool.tile([P, 4, FW], F16, tag="OFY")
            nc.vector.tensor_tensor(out=YL, in0=CY[:, 0:4, :], in1=CY[:, 1:5, :], op=OP.min)
            nc.vector.tensor_tensor(out=YH, in0=CY[:, 0:4, :], in1=CY[:, 1:5, :], op=OP.max)
            nc.vector.tensor_tensor(out=YL, in0=YL, in1=bc(NAB, 1, 4), op=OP.max)
            nc.vector.tensor_tensor(out=YH, in0=YH, in1=bc(HV, 3, 4), op=OP.min)
            nc.vector.tensor_tensor(out=YH, in0=YH, in1=YL, op=OP.max)

            T1 = pool.tile([P, 4, FW], F16, tag="T1")
            T2 = pool.tile([P, 4, FW], F16, tag="T2")
            nc.vector.tensor_tensor(out=T1, in0=YL, in1=CY[:, 0:4, :], op=OP.subtract)
            nc.vector.tensor_tensor(out=T2, in0=YH, in1=CY[:, 0:4, :], op=OP.subtract)
            rp_pat = _ap(RP2, 0, [(0, 2), (1, 2)])
            XLO = pool.tile([P, 4, FW], F16, tag="XLO")
            XHI = pool.tile([P, 4, FW], F16, tag="XHI")
            nc.vector.tensor_tensor(out=XLO, in0=T1, in1=rp_pat, op=OP.mult)
            nc.vector.tensor_tensor(out=XLO, in0=XLO, in1=CX, op=OP.add)
            nc.vector.tensor_tensor(out=XHI, in0=T2, in1=rp_pat, op=OP.mult)
            nc.vector.tensor_tensor(out=XHI, in0=XHI, in1=CX, op=OP.add)

            CA = pool.tile([P, 4, FW], F16, tag="CA")
            CB = pool.tile([P, 4, FW], F16, tag="CB")
            nc.vector.tensor_tensor(out=CA, in0=XLO, in1=bc(HV, 2, 4), op=OP.min)
            nc.vector.tensor_tensor(out=CA, in0=CA, in1=bc(NAB, 0, 4), op=OP.max)
            nc.vector.tensor_tensor(out=CB, in0=XHI, in1=bc(HV, 2, 4), op=OP.min)
            nc.vector.tensor_tensor(out=CB, in0=CB, in1=bc(NAB, 0, 4), op=OP.max)

            # Phi diff: (cb*xhi - ca*xlo) - 0.5*(cb-ca)*(cb+ca)
            PA = pool.tile([P, 4, FW], F16, tag="T1")     # reuse T1
            PB = pool.tile([P, 4, FW], F16, tag="T2")     # reuse T2
            nc.vector.tensor_tensor(out=PA, in0=CA, in1=XLO, op=OP.mult)
            nc.vector.tensor_tensor(out=PB, in0=CB, in1=XHI, op=OP.mult)
            D1 = pool.tile([P, 4, FW], F16, tag="OFX")    # reuse (YL dead)
            nc.vector.tensor_tensor(out=D1, in0=PB, in1=PA, op=OP.subtract)
            DM = pool.tile([P, 4, FW], F16, tag="XLO")    # reuse XLO
            DP = pool.tile([P, 4, FW], F16, tag="XHI")    # reuse XHI
            nc.vector.tensor_tensor(out=DM, in0=CB, in1=CA, op=OP.subtract)
            nc.vector.tensor_tensor(out=DP, in0=CB, in1=CA, op=OP.add)
            DMDP = pool.tile([P, 4, FW], F16, tag="CA")   # reuse CA
            nc.vector.tensor_tensor(out=DMDP, in0=DM, in1=DP, op=OP.mult)
            C1 = pool.tile([P, 4, FW], F16, tag="CB")     # reuse CB
            nc.vector.tensor_tensor(out=C1, in0=MU4, in1=D1, op=OP.mult)
            C2T = pool.tile([P, 4, FW], F16, tag="T1")    # reuse (PA dead)
            nc.vector.tensor_tensor(out=C2T, in0=HMU4, in1=DMDP, op=OP.mult)
            CT4 = pool.tile([P, 4, FW], F16, tag="T2")    # reuse (PB dead)
            nc.vector.tensor_tensor(out=CT4, in0=C1, in1=C2T, op=OP.subtract)
            R2 = pool.tile([P, 2, FW], F16, tag="XY")     # reuse XY
            nc.vector.tensor_tensor(out=R2, in0=CT4[:, 0:2, :], in1=CT4[:, 2:4, :], op=OP.add)
            R1 = pool.tile([P, FW], F16, tag="DTH")       # reuse DTH
            nc.vector.tensor_tensor(out=R1, in0=R2[:, 0, :], in1=R2[:, 1, :], op=OP.add)
            INTER = pool.tile([P, FW], F16, tag="INTER")
            nc.scalar.activation(INTER, R1, AF.Abs)

            # ------- focal front (overlaps ACT PTT with iou/DL tail) -----
            MT = pool.tile([P, 10, FW], F16, tag="ET")    # reuse ET (dead after S5)
            for c in range(10):
                nc.vector.tensor_scalar(out=MT[:, c, :], in0=inp(20), scalar1=float(c),
                                        scalar2=None, op0=OP.is_equal)
            nc.vector.tensor_tensor(out=MT, in0=MT, in1=IN[:, 22:32, :], op=OP.mult)
            L5 = pool.tile([P, 5, FW], F16, tag="S5")     # reuse S5 (dead)
            nc.vector.tensor_tensor(out=L5, in0=MT[:, 0:5, :], in1=MT[:, 5:10, :], op=OP.add)
            L22 = pool.tile([P, 2, FW], F16, tag="S22")
            nc.vector.tensor_tensor(out=L22, in0=L5[:, 0:2, :], in1=L5[:, 2:4, :], op=OP.add)
            LT = pool.tile([P, FW], F16, tag="LT")
            nc.vector.tensor_tensor(out=LT, in0=L22[:, 0, :], in1=L22[:, 1, :], op=OP.add)
            nc.vector.tensor_tensor(out=LT, in0=LT, in1=L5[:, 4, :], op=OP.add)
            LNS = pool.tile([P, FW], F16, tag="U4")
            nc.scalar.activation(LNS, SS, AF.Ln)
            LPT = pool.tile([P, FW], F16, tag="LPT")
            nc.vector.tensor_tensor(out=LPT, in0=LT, in1=LNS, op=OP.subtract)
            PTT = pool.tile([P, FW], F16, tag="SS")       # reuse SS
            nc.scalar.activation(PTT, LPT, AF.Exp)
            MP = pool.tile([P, FW], F16, tag="MP")
            nc.vector.tensor_scalar(out=MP, in0=inp(20), scalar1=0.5, scalar2=None, op0=OP.is_gt)
            nc.vector.tensor_scalar(out=MP, in0=MP, scalar1=-0.5, scalar2=0.75,
                                    op0=OP.mult, op1=OP.add)

            # ------- iou = inter / max(4(lw+ab) - inter, 1e-7) -----------
            U4 = pool.tile([P, FW], F16, tag="JKV")       # reuse WIT
            nc.vector.tensor_scalar(out=U4, in0=US, scalar1=4.0, scalar2=None, op0=OP.mult)
            nc.vector.tensor_tensor(out=U4, in0=U4, in1=INTER, op=OP.subtract)
            UG = pool.tile([P, FW], F32, tag="F32B")      # reuse AC32
            nc.vector.tensor_scalar(out=UG, in0=U4, scalar1=1e-7, scalar2=None, op0=OP.max)
            RU = pool.tile([P, FW], F32, tag="F32C")      # reuse RAC
            nc.vector.reciprocal_approx_fast(out=RU, in_=UG)
            IOU = pool.tile([P, FW], F16, tag="IOU")
            nc.vector.tensor_tensor(out=IOU, in0=INTER, in1=RU, op=OP.mult)
            nc.vector.tensor_scalar(out=IOU, in0=IOU, scalar1=1.0, scalar2=None, op0=OP.min)

            # enclosing tail (f32)
            nc.vector.tensor_scalar(out=C2V, in0=C2V, scalar1=1e-7, scalar2=None, op0=OP.max)
            RC2 = pool.tile([P, FW], F32, tag="RC2")
            nc.vector.reciprocal_approx_fast(out=RC2, in_=C2V)
            DL = pool.tile([P, FW], F32, tag="DL")
            nc.vector.tensor_tensor(out=DL, in0=D2, in1=RC2, op=OP.mult)
            DLM = pool.tile([P, FW], F16, tag="US")       # reuse US
            nc.vector.tensor_tensor(out=DLM, in0=DL, in1=IOU, op=OP.subtract)
            WDL = pool.tile([P, FW], F16, tag="JKV")
            nc.vector.tensor_tensor(out=WDL, in0=DLM, in1=inp(21), op=OP.mult)
            nc.scalar.activation(JKA[:, 0, :], WDL, AF.Copy,
                                 accum_out=ACC[:, A_DIOU:A_DIOU + 1])

            # ------- focal tail ------------------------------------------
            OM = pool.tile([P, FW], F16, tag="LT")        # reuse LT
            nc.vector.tensor_scalar(out=OM, in0=PTT, scalar1=-1.0, scalar2=1.0,
                                    op0=OP.mult, op1=OP.add)
            F1 = pool.tile([P, FW], F16, tag="INTER")     # reuse INTER
            nc.vector.tensor_tensor(out=F1, in0=OM, in1=LPT, op=OP.mult)
            nc.vector.tensor_tensor(out=F1, in0=F1, in1=MP, op=OP.mult)
            F2 = pool.tile([P, FW], F16, tag="JKV")
            nc.vector.tensor_tensor(out=F2, in0=F1, in1=OM, op=OP.mult)
            nc.scalar.activation(JKA[:, 0, :], F2, AF.Copy, scale=-1.0,
                                 accum_out=ACC[:, A_FOC:A_FOC + 1])

            # ---------- cross-partition reduce + output ----------
            PS = ppool.tile([1, 32], F32)
            nc.tensor.matmul(PS, ones, ACC, start=True, stop=True)
            OUT = spool.tile([1, 32], F32)
            nc.scalar.copy(out=OUT, in_=PS)
            nc.sync.dma_start(out=outp[:, :], in_=OUT)
    nc.compile()
    return nc


_NC_CACHE = None


def _get_nc():
    global _NC_CACHE
    if _NC_CACHE is None:
        _NC_CACHE = build_bass()
    return _NC_CACHE


def pack_inputs(cls_pred, reg_pred, iou_pred, reg_targets, iou_targets,
                cls_targets, reg_weights):
    """Returns list of 8 per-core input dicts."""
    B = cls_pred.shape[0]
    maps = []
    for b in range(B):
        h = np.empty((NSLOT, P, FW), np.float16)
        rp = np.asarray(reg_pred[b], np.float32).reshape(9, P, FW)
        rt = np.asarray(reg_targets[b], np.float32).reshape(9, P, FW)
        h[0] = rp[6]; h[1] = rt[6]                      # yaws
        h[2] = rp[0]; h[3] = rp[1]; h[4] = rt[0]; h[5] = rt[1]   # centers
        h[6] = rp[3]; h[7] = rp[4]; h[8] = rt[3]; h[9] = rt[4]   # w3, l4
        h[10] = rp[2]; h[11] = rp[5]; h[12] = rp[7]; h[13] = rp[8]  # z h vx vy
        h[14] = rt[2]; h[15] = rt[5]; h[16] = rt[7]; h[17] = rt[8]
        h[18] = np.asarray(iou_pred[b], np.float32).reshape(P, FW)
        h[19] = np.asarray(iou_targets[b], np.float32).reshape(P, FW)
        h[20] = np.asarray(cls_targets[b]).astype(np.float32).reshape(P, FW)
        h[21] = np.asarray(reg_weights[b]).astype(np.float32).reshape(P, FW)
        h[22:32] = np.asarray(cls_pred[b], np.float32).reshape(10, P, FW)
        maps.append({"h16": np.ascontiguousarray(h.transpose(1, 0, 2))})
    return maps


def partials_from_acc(acc):
    """acc: raw [1,32] per-core sums -> golden-style 9 partials."""
    a = np.asarray(acc, np.float64).reshape(32)
    w_s = a[A_W]
    focal_s = a[A_FOC]
    diou_s = a[A_DIOU]
    z_s = 0.5 * (a[A_ZSQ] - a[A_ZRL]) + 0.5 * w_s
    h_s = 0.5 * (a[A_HSQ] - a[A_HRL]) + 0.5 * w_s
    vel_s = 0.5 * (a[A_VSQ] - a[A_VRL]) + w_s          # vx_s + vy_s
    bce_s = a[A_WSP] - a[A_WIT]
    return np.array([focal_s, 65536.0, diou_s, z_s, h_s, 0.5 * vel_s,
                     0.5 * vel_s, bce_s, w_s])


def combine(parts):
    """parts: [8, 1, 32] per-core raw sums -> final [7] float32."""
    a = np.asarray(parts, np.float64).sum(0).reshape(32)
    w_s = max(a[A_W], 1.0)
    n_valid = 8.0 * 65536.0
    cls_loss = a[A_FOC] / n_valid
    bev_loss = (a[A_DIOU] + a[A_W]) / w_s
    z_loss = 0.5 * (a[A_ZSQ] - a[A_ZRL]) / w_s
    h_loss = 0.5 * (a[A_HSQ] - a[A_HRL]) / w_s
    vel_loss = 0.5 * (a[A_VSQ] - a[A_VRL]) / w_s
    iou_loss = (a[A_WSP] - a[A_WIT]) / w_s
    total = cls_loss + 2.0 * bev_loss + z_loss + h_loss + vel_loss + iou_loss
    return np.array([total, cls_loss, bev_loss, z_loss, h_loss, vel_loss, iou_loss],
                    np.float32)


def kernel(cls_pred, reg_pred, iou_pred, reg_targets, iou_targets,
           cls_targets, reg_weights, _trace=False):
    cls_pred, reg_pred, iou_pred, reg_targets, iou_targets, cls_targets, reg_weights = (
        np.asarray(a) for a in (cls_pred, reg_pred, iou_pred, reg_targets,
                                iou_targets, cls_targets, reg_weights))
    nc = _get_nc()
    in_maps = pack_inputs(cls_pred, reg_pred, iou_pred, reg_targets,
                          iou_targets, cls_targets, reg_weights)
    res = run_bass_kernel_spmd(nc, in_maps, core_ids=list(range(8)), trace=_trace)
    parts = [res.results[i]["out"] for i in range(8)]
    out = combine(parts)
    if _trace:
        return out, res
    return out


# revision 10
# speedup vs baseline: 1.7177x; 1.1093x over previous
"""DetectionBEVLoss Trainium2 kernel: 8-core data-parallel (1 batch/core).

Layout: per core 65536 sites as [128 partitions, 512 free]. Host packs all
inputs into one fp16 array [128, 32, 512] per core (slot map below), with
yaws first so trig starts as soon as the first small DMA piece lands.

Rotated intersection via a single-direction Green's-theorem identity:
  area(A ∩ B) = sum over A's 4 edges of  int clamp_a(x(y)) dy
with the y-integral restricted to |y| <= b (B = target box, axis-aligned
in its own frame, half-dims a x b; A = pred box with corners in B's
frame). Each edge needs one y-interval clip and a clamp antiderivative
Phi(x) = c*x - c^2/2 at the two clipped endpoints; the only reciprocals
are 1/|cos dth| and 1/|sin dth| (clamped at 2^-6, with the perturbation
applied consistently to the corner geometry).

Trig runs on the Scalar (ACT) engine via the sin table (cos = sin(x+pi/2)).
Smooth-L1 uses sl1(d)+0.5 = 0.5 d^2 + 0.5 - 0.5 relu(|d|-1)^2 so the sums
are plain Square-accumulates on ACT. BCE uses softplus(x) - x*t. The Pool
engine carries the enclosing-box chain, union/areas, d^2, and the softmax
sum tree.
"""
import math

import numpy as np

import concourse.bacc as bacc
import concourse.bass as bass
import concourse.mybir as mybir
import concourse.tile as tile
from concourse.bass_utils import run_bass_kernel_spmd

F16 = mybir.dt.float16
F32 = mybir.dt.float32
OP = mybir.AluOpType
AF = mybir.ActivationFunctionType

P = 128          # partitions
FW = 512         # free width per partition (128*512 = 65536 sites/core)

# slot map in the packed fp16 input [128, 32, 512]
# 0: yawp, 1: yawt | 2: xp, 3: yp, 4: xt, 5: yt | 6: w3p, 7: l4p, 8: w3t,
# 9: l4t | 10: zp, 11: hp, 12: vxp, 13: vyp | 14: zt, 15: ht, 16: vxt,
# 17: vyt | 18: iou_pred, 19: iou_targets, 20: cls_t, 21: w | 22-31: cls_pred
NSLOT = 32

EPSC = 2.0 ** -6     # |cos|,|sin| clamp (slope guard)

# ACC slots (fp32 [P,32])
A_FOC, A_DIOU, A_ZSQ, A_ZRL, A_HSQ, A_HRL, A_VSQ, A_VRL, A_WSP, A_WIT, A_W = range(11)


def _ap(t, s0, slot_dims, col0=0, ncol=FW, colstep=1):
    """Manual AP into tile t ([128, S, W]): base slot s0, then
    (slot_step, count) dims, innermost column dim."""
    ss = t.ap[-2][0]
    ap = [list(t.ap[0])] + [[s * ss, c] for s, c in slot_dims] + [[colstep, ncol]]
    return bass.AP(tensor=t.tensor, offset=t.offset + s0 * ss + col0, ap=ap)


def build_bass():
    nc = bacc.Bacc("TRN2", target_bir_lowering=False, debug=False)
    h16 = nc.declare_dram_parameter("h16", [P, NSLOT, FW], F16, isOutput=False)
    outp = nc.declare_dram_parameter("out", [1, 32], F32, isOutput=True)

    with tile.TileContext(nc) as tc:
        with (
            tc.tile_pool(name="main", bufs=1) as pool,
            tc.tile_pool(name="small", bufs=1) as spool,
            tc.tile_pool(name="ps", bufs=1, space="PSUM") as ppool,
        ):
            IN = pool.tile([P, NSLOT, FW], F16)
            nc.sync.dma_start(out=IN[:, 0:2, :], in_=h16[:, 0:2, :])
            nc.sync.dma_start(out=IN[:, 2:10, :], in_=h16[:, 2:10, :])
            nc.sync.dma_start(out=IN[:, 10:22, :], in_=h16[:, 10:22, :])
            nc.sync.dma_start(out=IN[:, 22:32, :], in_=h16[:, 22:32, :])

            ones = spool.tile([P, 1], F32)
            nc.vector.memset(ones, 1.0)
            ACC = spool.tile([P, 32], F32)
            nc.vector.memset(ACC, 0.0)
            PIH = spool.tile([P, 1], F32)
            nc.vector.memset(PIH, math.pi / 2)
            NEG1 = spool.tile([P, 1], F32)
            nc.vector.memset(NEG1, -1.0)
            TINY = spool.tile([P, 1], F32)
            nc.vector.memset(TINY, 1e-6)

            def inp(s):
                return IN[:, s, :]

            def bc(t, s, n):  # broadcast slot s of tile t over n slots
                return _ap(t, s, [(0, n)])

            # ================= head ======================================
            DTH = pool.tile([P, FW], F16, tag="DTH")
            nc.vector.tensor_tensor(out=DTH, in0=inp(0), in1=inp(1), op=OP.subtract)

            # TRG slots: 0:cp 1:sp 2:ct 3:st 4:cd 5:sd
            TRG = pool.tile([P, 6, FW], F16, tag="TRG")
            nc.scalar.activation(TRG[:, 5, :], DTH, AF.Sin)
            nc.scalar.activation(TRG[:, 4, :], DTH, AF.Sin, bias=PIH[:, :])
            SGN4 = pool.tile([P, 4, FW], F16, tag="SGN4")  # scd ssd -scd -ssd
            nc.scalar.activation(SGN4[:, 0:2, :], TRG[:, 4:6, :], AF.Sign, bias=TINY[:, :])
            nc.scalar.activation(TRG[:, 3, :], inp(1), AF.Sin)
            nc.scalar.activation(TRG[:, 2, :], inp(1), AF.Sin, bias=PIH[:, :])
            nc.scalar.activation(TRG[:, 1, :], inp(0), AF.Sin)
            nc.scalar.activation(TRG[:, 0, :], inp(0), AF.Sin, bias=PIH[:, :])
            A6 = pool.tile([P, 6, FW], F16, tag="A6")  # |cp| |sp| |ct| |st| |cd| |sd|
            nc.scalar.activation(A6, TRG, AF.Abs)

            # DVE head preps (only need DMA pieces 1-2)
            HV = pool.tile([P, 4, FW], F16, tag="HV")     # l w a b (half dims)
            nc.vector.tensor_scalar(out=_ap(HV, 1, [(-1, 2)]), in0=IN[:, 6:8, :],
                                    scalar1=0.5, scalar2=None, op0=OP.mult)
            nc.vector.tensor_scalar(out=_ap(HV, 3, [(-1, 2)]), in0=IN[:, 8:10, :],
                                    scalar1=0.5, scalar2=None, op0=OP.mult)
            NAB = pool.tile([P, 2, FW], F16, tag="NAB")   # -a -b
            nc.vector.tensor_scalar(out=NAB, in0=HV[:, 2:4, :],
                                    scalar1=-1.0, scalar2=None, op0=OP.mult)
            DXY = pool.tile([P, 2, FW], F16, tag="DXY")   # dx dy (world)
            nc.vector.tensor_tensor(out=DXY, in0=IN[:, 2:4, :], in1=IN[:, 4:6, :],
                                    op=OP.subtract)
            AD2 = pool.tile([P, 2, FW], F16, tag="AD2")   # |dx| |dy|
            nc.scalar.activation(AD2, DXY, AF.Abs)

            # --- SL1 preps early (ACT consumes during geometry) -----------
            D4 = pool.tile([P, 4, FW], F16, tag="T8")     # dz dh dvx dvy
            nc.vector.tensor_tensor(out=D4, in0=IN[:, 10:14, :], in1=IN[:, 14:18, :],
                                    op=OP.subtract)
            WD4 = pool.tile([P, 4, FW], F16, tag="WD4")
            nc.vector.tensor_tensor(out=WD4, in0=D4, in1=bc(IN, 21, 4), op=OP.mult)
            RL4 = pool.tile([P, 4, FW], F16, tag="RL4")   # relu(|wd|-1)
            nc.scalar.activation(RL4, WD4, AF.Abs)
            nc.scalar.activation(RL4, RL4, AF.Relu, bias=NEG1[:, :])
            JKA = pool.tile([P, 2, FW], F16, tag="JKA")   # ACT junk sink
            nc.scalar.activation(JKA[:, 0, :], WD4[:, 0, :], AF.Square,
                                 accum_out=ACC[:, A_ZSQ:A_ZSQ + 1])
            nc.scalar.activation(JKA[:, 0, :], RL4[:, 0, :], AF.Square,
                                 accum_out=ACC[:, A_ZRL:A_ZRL + 1])
            nc.scalar.activation(JKA[:, 0, :], WD4[:, 1, :], AF.Square,
                                 accum_out=ACC[:, A_HSQ:A_HSQ + 1])
            nc.scalar.activation(JKA[:, 0, :], RL4[:, 1, :], AF.Square,
                                 accum_out=ACC[:, A_HRL:A_HRL + 1])
            nc.scalar.activation(JKA, WD4[:, 2:4, :], AF.Square,
                                 accum_out=ACC[:, A_VSQ:A_VSQ + 1])
            nc.scalar.activation(JKA, RL4[:, 2:4, :], AF.Square,
                                 accum_out=ACC[:, A_VRL:A_VRL + 1])
            nc.scalar.activation(JKA[:, 0, :], inp(21), AF.Copy,
                                 accum_out=ACC[:, A_W:A_W + 1])

            # --- enclosing-box + union heads: feed the Pool engine --------
            # E8 = (l|cp|, l|sp|, w|cp|, w|sp|, a|ct|, a|st|, b|ct|, b|st|)
            E8 = pool.tile([P, 8, FW], F16, tag="E8")
            nc.vector.tensor_tensor(out=E8[:, 0:4, :],
                                    in0=_ap(HV, 0, [(1, 2), (0, 2)]),
                                    in1=_ap(A6, 0, [(0, 2), (1, 2)]), op=OP.mult)
            nc.vector.tensor_tensor(out=E8[:, 4:8, :],
                                    in0=_ap(HV, 2, [(1, 2), (0, 2)]),
                                    in1=_ap(A6, 2, [(0, 2), (1, 2)]), op=OP.mult)
            # EXY = (ex_p, ey_p, ex_t, ey_t): ex = l|c|+w|s| ; ey = l|s|+w|c|
            EXY = pool.tile([P, 4, FW], F16, tag="EXY")
            nc.vector.tensor_tensor(out=EXY, in0=_ap(E8, 0, [(4, 2), (1, 2)]),
                                    in1=_ap(E8, 3, [(4, 2), (-1, 2)]), op=OP.add)
            UAB = pool.tile([P, 2, FW], F16, tag="UAB")   # lw ab
            nc.vector.tensor_tensor(out=UAB, in0=_ap(HV, 0, [(2, 2)]),
                                    in1=_ap(HV, 1, [(2, 2)]), op=OP.mult)
            US = pool.tile([P, FW], F16, tag="US")        # lw+ab
            nc.vector.tensor_tensor(out=US, in0=UAB[:, 0, :], in1=UAB[:, 1, :], op=OP.add)
            DEL = pool.tile([P, 2, FW], F16, tag="DEL")
            SUM = pool.tile([P, 2, FW], F16, tag="SUM")
            nc.vector.tensor_tensor(out=DEL, in0=EXY[:, 0:2, :], in1=EXY[:, 2:4, :],
                                    op=OP.subtract)
            nc.vector.tensor_tensor(out=SUM, in0=EXY[:, 0:2, :], in1=EXY[:, 2:4, :],
                                    op=OP.add)
            ADL = pool.tile([P, 2, FW], F16, tag="ADL")
            nc.scalar.activation(ADL, DEL, AF.Abs)
            MXD = pool.tile([P, 2, FW], F16, tag="DEL")   # reuse DEL
            nc.vector.tensor_tensor(out=MXD, in0=ADL, in1=AD2, op=OP.max)
            W2 = pool.tile([P, 2, FW], F16, tag="ADL")    # reuse ADL
            nc.vector.tensor_tensor(out=W2, in0=SUM, in1=MXD, op=OP.add)
            WSQ = pool.tile([P, 2, FW], F32, tag="F32A")
            nc.vector.tensor_tensor(out=WSQ, in0=W2, in1=W2, op=OP.mult)
            C2V = pool.tile([P, FW], F32, tag="C2V")
            nc.vector.tensor_tensor(out=C2V, in0=WSQ[:, 0, :], in1=WSQ[:, 1, :], op=OP.add)
            DD2 = pool.tile([P, 2, FW], F16, tag="SUM")   # reuse SUM
            nc.vector.tensor_tensor(out=DD2, in0=DXY, in1=DXY, op=OP.mult)
            D2 = pool.tile([P, FW], F32, tag="D2")
            nc.vector.tensor_tensor(out=D2, in0=DD2[:, 0, :], in1=DD2[:, 1, :], op=OP.add)

            # --- BCE (ACT + small DVE) ------------------------------------
            SP = pool.tile([P, FW], F16, tag="SP")
            nc.scalar.activation(SP, inp(18), AF.Exp)
            nc.scalar.activation(SP, SP, AF.Ln, bias=1.0)
            WSP = pool.tile([P, FW], F16, tag="WSP")
            nc.vector.tensor_tensor(out=WSP, in0=SP, in1=inp(21), op=OP.mult)
            nc.scalar.activation(JKA[:, 0, :], WSP, AF.Copy,
                                 accum_out=ACC[:, A_WSP:A_WSP + 1])
            WIP = pool.tile([P, FW], F16, tag="WIP")
            nc.vector.tensor_tensor(out=WIP, in0=inp(18), in1=inp(21), op=OP.mult)
            WIT = pool.tile([P, FW], F16, tag="JKV")
            nc.vector.tensor_tensor(out=WIT, in0=WIP, in1=inp(19), op=OP.mult)
            nc.scalar.activation(JKA[:, 0, :], WIT, AF.Copy,
                                 accum_out=ACC[:, A_WIT:A_WIT + 1])

            # focal exps early on ACT (needs DMA piece 4)
            ET = pool.tile([P, 10, FW], F16, tag="ET")
            nc.scalar.activation(ET, IN[:, 22:32, :], AF.Exp)
            # softmax-sum tree on Pool
            S5 = pool.tile([P, 5, FW], F16, tag="S5")
            nc.vector.tensor_tensor(out=S5, in0=ET[:, 0:5, :], in1=ET[:, 5:10, :], op=OP.add)
            S22 = pool.tile([P, 2, FW], F16, tag="S22")
            nc.vector.tensor_tensor(out=S22, in0=S5[:, 0:2, :], in1=S5[:, 2:4, :], op=OP.add)
            SS = pool.tile([P, FW], F16, tag="SS")
            nc.vector.tensor_tensor(out=SS, in0=S22[:, 0, :], in1=S22[:, 1, :], op=OP.add)
            nc.vector.tensor_tensor(out=SS, in0=SS, in1=S5[:, 4, :], op=OP.add)

            # ============== geometry: clamped trig ========================
            AC2 = pool.tile([P, 2, FW], F16, tag="AC2")   # |cd|' |sd|' clamped
            nc.vector.tensor_scalar(out=AC2, in0=A6[:, 4:6, :], scalar1=EPSC,
                                    scalar2=None, op0=OP.max)
            CS2 = pool.tile([P, 2, FW], F16, tag="CS2")   # c~ s~
            nc.vector.tensor_tensor(out=CS2, in0=SGN4[:, 0:2, :], in1=AC2, op=OP.mult)
            AC32 = pool.tile([P, 2, FW], F32, tag="F32B")
            nc.vector.tensor_copy(out=AC32, in_=AC2)
            RAC = pool.tile([P, 2, FW], F32, tag="F32C")  # 1/|c|' 1/|s|'
            nc.vector.reciprocal_approx_fast(out=RAC.rearrange("p a b -> p (a b)"),
                                             in_=AC32.rearrange("p a b -> p (a b)"))
            nc.vector.tensor_scalar(out=SGN4[:, 2:4, :], in0=SGN4[:, 0:2, :],
                                    scalar1=-1.0, scalar2=None, op0=OP.mult)
            Q2S = pool.tile([P, 2, FW], F16, tag="Q2S")   # (c*ss, -s*sc)
            nc.vector.tensor_tensor(out=Q2S, in0=CS2, in1=SGN4[:, 1:3, :], op=OP.mult)
            RP2 = pool.tile([P, 2, FW], F16, tag="RP2")   # (c/s, -s/c)
            nc.vector.tensor_tensor(out=RP2, in0=Q2S, in1=_ap(RAC, 1, [(-1, 2)]), op=OP.mult)
            MU2 = pool.tile([P, 2, FW], F16, tag="Q2S")   # |s|/|c|, |c|/|s| (reuse)
            nc.vector.tensor_tensor(out=MU2, in0=_ap(AC2, 1, [(-1, 2)]), in1=RAC, op=OP.mult)
            MU4 = pool.tile([P, 4, FW], F16, tag="MU4")   # -m1 -m2 m1 m2 (signed)
            nc.vector.tensor_tensor(out=MU4[:, 2:4, :], in0=MU2, in1=SGN4[:, 0:2, :], op=OP.mult)
            nc.vector.tensor_scalar(out=MU4[:, 0:2, :], in0=MU4[:, 2:4, :],
                                    scalar1=-1.0, scalar2=None, op0=OP.mult)
            HMU4 = pool.tile([P, 4, FW], F16, tag="HMU4")
            nc.vector.tensor_scalar(out=HMU4, in0=MU4, scalar1=0.5, scalar2=None, op0=OP.mult)

            # A center in B frame: X = ct*dx+st*dy ; Y = ct*dy-st*dx
            RP5 = pool.tile([P, 5, FW], F16, tag="RP5")
            nc.vector.tensor_tensor(out=RP5[:, 0:4, :],
                                    in0=_ap(DXY, 0, [(0, 2), (1, 2)]),
                                    in1=_ap(TRG, 2, [(1, 2), (0, 2)]), op=OP.mult)
            nc.vector.tensor_scalar(out=RP5[:, 4, :], in0=RP5[:, 2, :],
                                    scalar1=-1.0, scalar2=None, op0=OP.mult)
            XY = pool.tile([P, 2, FW], F16, tag="XY")
            nc.vector.tensor_tensor(out=XY, in0=RP5[:, 0:2, :],
                                    in1=_ap(RP5, 3, [(1, 2)]), op=OP.add)

            # corner offsets: T8 = (lc, ls, wc, ws, -lc, -ls, -wc, -ws)
            T8 = pool.tile([P, 8, FW], F16, tag="T8")     # reuse D4 buffer
            nc.vector.tensor_tensor(out=T8[:, 0:4, :],
                                    in0=_ap(HV, 0, [(1, 2), (0, 2)]),
                                    in1=_ap(CS2, 0, [(0, 2), (1, 2)]), op=OP.mult)
            nc.vector.tensor_scalar(out=T8[:, 4:8, :], in0=T8[:, 0:4, :],
                                    scalar1=-1.0, scalar2=None, op0=OP.mult)
            OFX = pool.tile([P, 4, FW], F16, tag="OFX")
            nc.vector.tensor_tensor(out=OFX[:, 0:2, :], in0=_ap(T8, 0, [(4, 2)]),
                                    in1=_ap(T8, 3, [(0, 2)]), op=OP.add)
            nc.vector.tensor_scalar(out=OFX[:, 2:4, :], in0=OFX[:, 0:2, :],
                                    scalar1=-1.0, scalar2=None, op0=OP.mult)
            OFY = pool.tile([P, 4, FW], F16, tag="OFY")
            nc.vector.tensor_tensor(out=OFY[:, 0:2, :], in0=_ap(T8, 1, [(4, 2)]),
                                    in1=_ap(T8, 6, [(0, 2)]), op=OP.add)
            nc.vector.tensor_scalar(out=OFY[:, 2:4, :], in0=OFY[:, 0:2, :],
                                    scalar1=-1.0, scalar2=None, op0=OP.mult)
            CX = pool.tile([P, 4, FW], F16, tag="CX")
            nc.vector.tensor_tensor(out=CX, in0=bc(XY, 0, 4), in1=OFX, op=OP.add)
            CY = pool.tile([P, 5, FW], F16, tag="RP5")    # reuse RP5
            nc.vector.tensor_tensor(out=CY[:, 0:4, :], in0=bc(XY, 1, 4), in1=OFY, op=OP.add)
            nc.vector.tensor_copy(out=CY[:, 4, :], in_=CY[:, 0, :])

            # y-interval clip per edge (YL/YH reuse OFX/OFY buffers)
            YL = pool.tile([P, 4, FW], F16, tag="OFX")
            YH = pool.tile([P, 4, FW], F16, tag="OFY")
            nc.vector.tensor_tensor(out=YL, in0=CY[:, 0:4, :], in1=CY[:, 1:5, :], op=OP.min)
            nc.vector.tensor_tensor(out=YH, in0=CY[:, 0:4, :], in1=CY[:, 1:5, :], op=OP.max)
            nc.vector.tensor_tensor(out=YL, in0=YL, in1=bc(NAB, 1, 4), op=OP.max)
            nc.vector.tensor_tensor(out=YH, in0=YH, in1=bc(HV, 3, 4), op=OP.min)
            nc.vector.tensor_tensor(out=YH, in0=YH, in1=YL, op=OP.max)

            T1 = pool.tile([P, 4, FW], F16, tag="T1")
            T2 = pool.tile([P, 4, FW], F16, tag="T2")
            nc.vector.tensor_tensor(out=T1, in0=YL, in1=CY[:, 0:4, :], op=OP.subtract)
            nc.vector.tensor_tensor(out=T2, in0=YH, in1=CY[:, 0:4, :], op=OP.subtract)
            rp_pat = _ap(RP2, 0, [(0, 2), (1, 2)])
            XLO = pool.tile([P, 4, FW], F16, tag="XLO")
            XHI = pool.tile([P, 4, FW], F16, tag="XHI")
            nc.vector.tensor_tensor(out=XLO, in0=T1, in1=rp_pat, op=OP.mult)
            nc.vector.tensor_tensor(out=XLO, in0=XLO, in1=CX, op=OP.add)
            nc.vector.tensor_tensor(out=XHI, in0=T2, in1=rp_pat, op=OP.mult)
            nc.vector.tensor_tensor(out=XHI, in0=XHI, in1=CX, op=OP.add)

            CA = pool.tile([P, 4, FW], F16, tag="CA")
            CB = pool.tile([P, 4, FW], F16, tag="CB")
            nc.vector.tensor_tensor(out=CA, in0=XLO, in1=bc(HV, 2, 4), op=OP.min)
            nc.vector.tensor_tensor(out=CA, in0=CA, in1=bc(NAB, 0, 4), op=OP.max)
            nc.vector.tensor_tensor(out=CB, in0=XHI, in1=bc(HV, 2, 4), op=OP.min)
            nc.vector.tensor_tensor(out=CB, in0=CB, in1=bc(NAB, 0, 4), op=OP.max)

            # Phi diff: (cb*xhi - ca*xlo) - 0.5*(cb-ca)*(cb+ca)
            PA = pool.tile([P, 4, FW], F16, tag="T1")     # reuse T1
            PB = pool.tile([P, 4, FW], F16, tag="T2")     # reuse T2
            nc.vector.tensor_tensor(out=PA, in0=CA, in1=XLO, op=OP.mult)
            nc.vector.tensor_tensor(out=PB, in0=CB, in1=XHI, op=OP.mult)
            D1 = pool.tile([P, 4, FW], F16, tag="OFX")    # reuse (YL dead)
            nc.vector.tensor_tensor(out=D1, in0=PB, in1=PA, op=OP.subtract)
            DM = pool.tile([P, 4, FW], F16, tag="XLO")    # reuse XLO
            DP = pool.tile([P, 4, FW], F16, tag="XHI")    # reuse XHI
            nc.vector.tensor_tensor(out=DM, in0=CB, in1=CA, op=OP.subtract)
            nc.vector.tensor_tensor(out=DP, in0=CB, in1=CA, op=OP.add)
            DMDP = pool.tile([P, 4, FW], F16, tag="CA")   # reuse CA
            nc.vector.tensor_tensor(out=DMDP, in0=DM, in1=DP, op=OP.mult)
            C1 = pool.tile([P, 4, FW], F16, tag="CB")     # reuse CB
            nc.vector.tensor_tensor(out=C1, in0=MU4, in1=D1, op=OP.mult)
            C2T = pool.tile([P, 4, FW], F16, tag="T1")    # reuse (PA dead)
            nc.vector.tensor_tensor(out=C2T, in0=HMU4, in1=DMDP, op=OP.mult)
            CT4 = pool.tile([P, 4, FW], F16, tag="T2")    # reuse (PB dead)
            nc.vector.tensor_tensor(out=CT4, in0=C1, in1=C2T, op=OP.subtract)
            R2 = pool.tile([P, 2, FW], F16, tag="XY")     # reuse XY
            nc.vector.tensor_tensor(out=R2, in0=CT4[:, 0:2, :], in1=CT4[:, 2:4, :], op=OP.add)
            R1 = pool.tile([P, FW], F16, tag="DTH")       # reuse DTH
            nc.vector.tensor_tensor(out=R1, in0=R2[:, 0, :], in1=R2[:, 1, :], op=OP.add)
            INTER = pool.tile([P, FW], F16, tag="INTER")
            nc.scalar.activation(INTER, R1, AF.Abs)

            # ------- focal front (overlaps ACT PTT with iou/DL tail) -----
            MT = pool.tile([P, 10, FW], F16, tag="ET")    # reuse ET (dead after S5)
            for c in range(10):
                nc.vector.tensor_scalar(out=MT[:, c, :], in0=inp(20), scalar1=float(c),
                                        scalar2=None, op0=OP.is_equal)
            nc.vector.tensor_tensor(out=MT, in0=MT, in1=IN[:, 22:32, :], op=OP.mult)
            L5 = pool.tile([P, 5, FW], F16, tag="S5")     # reuse S5 (dead)
            nc.vector.tensor_tensor(out=L5, in0=MT[:, 0:5, :], in1=MT[:, 5:10, :], op=OP.add)
            L22 = pool.tile([P, 2, FW], F16, tag="S22")
            nc.vector.tensor_tensor(out=L22, in0=L5[:, 0:2, :], in1=L5[:, 2:4, :], op=OP.add)
            LT = pool.tile([P, FW], F16, tag="LT")
            nc.vector.tensor_tensor(out=LT, in0=L22[:, 0, :], in1=L22[:, 1, :], op=OP.add)
            nc.vector.tensor_tensor(out=LT, in0=LT, in1=L5[:, 4, :], op=OP.add)
            LNS = pool.tile([P, FW], F16, tag="U4")
            nc.scalar.activation(LNS, SS, AF.Ln)
            LPT = pool.tile([P, FW], F16, tag="LPT")
            nc.vector.tensor_tensor(out=LPT, in0=LT, in1=LNS, op=OP.subtract)
            PTT = pool.tile([P, FW], F16, tag="SS")       # reuse SS
            nc.scalar.activation(PTT, LPT, AF.Exp)
            MP = pool.tile([P, FW], F16, tag="MP")
            nc.vector.tensor_scalar(out=MP, in0=inp(20), scalar1=0.5, scalar2=None, op0=OP.is_gt)
            nc.vector.tensor_scalar(out=MP, in0=MP, scalar1=-0.5, scalar2=0.75,
                                    op0=OP.mult, op1=OP.add)

            # ------- iou = inter / max(4(lw+ab) - inter, 1e-7) -----------
            U4 = pool.tile([P, FW], F16, tag="JKV")       # reuse WIT
            nc.vector.tensor_scalar(out=U4, in0=US, scalar1=4.0, scalar2=None, op0=OP.mult)
            nc.vector.tensor_tensor(out=U4, in0=U4, in1=INTER, op=OP.subtract)
            UG = pool.tile([P, FW], F32, tag="F32B")      # reuse AC32
            nc.vector.tensor_scalar(out=UG, in0=U4, scalar1=1e-7, scalar2=None, op0=OP.max)
            RU = pool.tile([P, FW], F32, tag="F32C")      # reuse RAC
            nc.vector.reciprocal_approx_fast(out=RU, in_=UG)
            IOU = pool.tile([P, FW], F16, tag="IOU")
            nc.vector.tensor_tensor(out=IOU, in0=INTER, in1=RU, op=OP.mult)
            nc.vector.tensor_scalar(out=IOU, in0=IOU, scalar1=1.0, scalar2=None, op0=OP.min)

            # enclosing tail (f32)
            nc.vector.tensor_scalar(out=C2V, in0=C2V, scalar1=1e-7, scalar2=None, op0=OP.max)
            RC2 = pool.tile([P, FW], F32, tag="RC2")
            nc.vector.reciprocal_approx_fast(out=RC2, in_=C2V)
            DL = pool.tile([P, FW], F32, tag="DL")
            nc.vector.tensor_tensor(out=DL, in0=D2, in1=RC2, op=OP.mult)
            DLM = pool.tile([P, FW], F16, tag="US")       # reuse US
            nc.vector.tensor_tensor(out=DLM, in0=DL, in1=IOU, op=OP.subtract)
            WDL = pool.tile([P, FW], F16, tag="JKV")
            nc.vector.tensor_tensor(out=WDL, in0=DLM, in1=inp(21), op=OP.mult)
            nc.scalar.activation(JKA[:, 0, :], WDL, AF.Copy,
                                 accum_out=ACC[:, A_DIOU:A_DIOU + 1])

            # ------- focal tail ------------------------------------------
            OM = pool.tile([P, FW], F16, tag="LT")        # reuse LT
            nc.vector.tensor_scalar(out=OM, in0=PTT, scalar1=-1.0, scalar2=1.0,
                                    op0=OP.mult, op1=OP.add)
            F1 = pool.tile([P, FW], F16, tag="INTER")     # reuse INTER
            nc.vector.tensor_tensor(out=F1, in0=OM, in1=LPT, op=OP.mult)
            nc.vector.tensor_tensor(out=F1, in0=F1, in1=MP, op=OP.mult)
            F2 = pool.tile([P, FW], F16, tag="JKV")
            nc.vector.tensor_tensor(out=F2, in0=F1, in1=OM, op=OP.mult)
            nc.scalar.activation(JKA[:, 0, :], F2, AF.Copy, scale=-1.0,
                                 accum_out=ACC[:, A_FOC:A_FOC + 1])

            # ---------- cross-partition reduce + output ----------
            PS = ppool.tile([1, 32], F32)
            nc.tensor.matmul(PS, ones, ACC, start=True, stop=True)
            OUT = spool.tile([1, 32], F32)
            nc.scalar.copy(out=OUT, in_=PS)
            nc.sync.dma_start(out=outp[:, :], in_=OUT)
    nc.compile()
    return nc


_NC_CACHE = None


def _get_nc():
    global _NC_CACHE
    if _NC_CACHE is None:
        _NC_CACHE = build_bass()
    return _NC_CACHE


def pack_inputs(cls_pred, reg_pred, iou_pred, reg_targets, iou_targets,
                cls_targets, reg_weights):
    """Returns list of 8 per-core input dicts."""
    B = cls_pred.shape[0]
    maps = []
    for b in range(B):
        h = np.empty((NSLOT, P, FW), np.float16)
        rp = np.asarray(reg_pred[b], np.float32).reshape(9, P, FW)
        rt = np.asarray(reg_targets[b], np.float32).reshape(9, P, FW)
        h[0] = rp[6]; h[1] = rt[6]                      # yaws
        h[2] = rp[0]; h[3] = rp[1]; h[4] = rt[0]; h[5] = rt[1]   # centers
        h[6] = rp[3]; h[7] = rp[4]; h[8] = rt[3]; h[9] = rt[4]   # w3, l4
        h[10] = rp[2]; h[11] = rp[5]; h[12] = rp[7]; h[13] = rp[8]  # z h vx vy
        h[14] = rt[2]; h[15] = rt[5]; h[16] = rt[7]; h[17] = rt[8]
        h[18] = np.asarray(iou_pred[b], np.float32).reshape(P, FW)
        h[19] = np.asarray(iou_targets[b], np.float32).reshape(P, FW)
        h[20] = np.asarray(cls_targets[b]).astype(np.float32).reshape(P, FW)
        h[21] = np.asarray(reg_weights[b]).astype(np.float32).reshape(P, FW)
        h[22:32] = np.asarray(cls_pred[b], np.float32).reshape(10, P, FW)
        maps.append({"h16": np.ascontiguousarray(h.transpose(1, 0, 2))})
    return maps


def partials_from_acc(acc):
    """acc: raw [1,32] per-core sums -> golden-style 9 partials."""
    a = np.asarray(acc, np.float64).reshape(32)
    w_s = a[A_W]
    focal_s = a[A_FOC]
    diou_s = a[A_DIOU]
    z_s = 0.5 * (a[A_ZSQ] - a[A_ZRL]) + 0.5 * w_s
    h_s = 0.5 * (a[A_HSQ] - a[A_HRL]) + 0.5 * w_s
    vel_s = 0.5 * (a[A_VSQ] - a[A_VRL]) + w_s          # vx_s + vy_s
    bce_s = a[A_WSP] - a[A_WIT]
    return np.array([focal_s, 65536.0, diou_s, z_s, h_s, 0.5 * vel_s,
                     0.5 * vel_s, bce_s, w_s])


def combine(parts):
    """parts: [8, 1, 32] per-core raw sums -> final [7] float32."""
    a = np.asarray(parts, np.float64).sum(0).reshape(32)
    w_s = max(a[A_W], 1.0)
    n_valid = 8.0 * 65536.0
    cls_loss = a[A_FOC] / n_valid
    bev_loss = (a[A_DIOU] + a[A_W]) / w_s
    z_loss = 0.5 * (a[A_ZSQ] - a[A_ZRL]) / w_s
    h_loss = 0.5 * (a[A_HSQ] - a[A_HRL]) / w_s
    vel_loss = 0.5 * (a[A_VSQ] - a[A_VRL]) / w_s
    iou_loss = (a[A_WSP] - a[A_WIT]) / w_s
    total = cls_loss + 2.0 * bev_loss + z_loss + h_loss + vel_loss + iou_loss
    return np.array([total, cls_loss, bev_loss, z_loss, h_loss, vel_loss, iou_loss],
                    np.float32)


def kernel(cls_pred, reg_pred, iou_pred, reg_targets, iou_targets,
           cls_targets, reg_weights, _trace=False):
    cls_pred, reg_pred, iou_pred, reg_targets, iou_targets, cls_targets, reg_weights = (
        np.asarray(a) for a in (cls_pred, reg_pred, iou_pred, reg_targets,
                                iou_targets, cls_targets, reg_weights))
    nc = _get_nc()
    in_maps = pack_inputs(cls_pred, reg_pred, iou_pred, reg_targets,
                          iou_targets, cls_targets, reg_weights)
    res = run_bass_kernel_spmd(nc, in_maps, core_ids=list(range(8)), trace=_trace)
    parts = [res.results[i]["out"] for i in range(8)]
    out = combine(parts)
    if _trace:
        return out, res
    return out
